# revision 12
# baseline (speedup 1.0000x reference)
"""TRN2 Bass kernel for 16-head causal MHA (B=4, T=2048, C=2048), fp32 in/out.

Sharding: 8 cores = 4 batches x 2 head-groups (8 heads each).  Each core
computes q/k/v projections for its head group on its batch (tensor-parallel
column split of Wq/Wk/Wv), causal attention in the S^T layout, and a partial
output projection with the row slice of Wp.  The two head-group partials per
batch are summed on the host, plus the output bias.

Design (v7, bf16 datapath):
- All operand data is bf16; every matmul accumulates in fp32 PSUM.
- Startup: DMA emission is ordered by first consumption.  The sync queue
  carries only the Q/K weight columns (first two waves' stationaries split
  so the first LDWEIGHTS fires after 64KB); x^T chunks ride the scalar +
  gpsimd queues in consumption order; ones/tri/wvg/wp follow later.
- Q^T/K^T via e-interleaved wave pairs (8 PSUM banks) as before; V is
  computed for t-tiles 0..7 in the projection phase, while t-tiles 8..15
  are DEFERRED and emitted as PE filler inside the qg=0/qg=1 attention
  stretch (which is otherwise exp-latency bound).  x^T is split into two
  tiles (t<1024 / t>=1024) so only the high half (+wv weights) stays
  resident during that overlap window.
- Attention (S^T layout, multiplicative causal mask on diagonal blocks,
  packed variable-width score/attn matmuls, ones-matmul softmax
  denominator broadcast, single-pass DVE reciprocal, gpsimd normalize)
  is unchanged except: atn lives in two qg-major tiles (qg0/1, qg2/3),
  and out-projection D-blocks are interleaved WITHIN qg2/qg3 cgroups
  (between score pairs) so exp latency never stalls the PE.
- Output projection column groups drain og-major so the og-sliced Wp DMA
  (issued as SBUF frees up after the overlap window) arrives just in time;
  y tiles DMA out round-robin across the sync/scalar/gpsimd queues.
"""
import math
import os
from collections import deque

import ml_dtypes
import numpy as np

import concourse.bass as bass
import concourse.tile as tile
from concourse import bacc, mybir
from concourse.bass_utils import run_bass_kernel_spmd

f32 = mybir.dt.float32
bf16 = mybir.dt.bfloat16
AF = mybir.ActivationFunctionType
BF = ml_dtypes.bfloat16

N_CORES = 8
HD = 128                      # head dim

# results of the last run_bass_kernel_spmd call (for test harness profiling)
LAST_RESULT = None


def build_nc(T=2048, E=2048, D=1024, NOD=2048, TG=512, bias=False,
             num_devices=N_CORES):
    """Build + compile the per-core Bass program."""
    NH = D // HD              # heads per core
    EC = E // 128 + (1 if bias else 0)
    TC = T // 128             # 128-row tiles along T
    TGC = T // TG             # q-groups
    ODG = NOD // 512          # out-proj column groups
    VN = 512                  # v-projection moving width
    NVS = D // VN
    TLO = T // 2              # t-split point: xt_lo covers t<TLO
    TTLO = TLO // 128         # t-tiles in the low half (V in phase B)
    scale = 1.0 / math.sqrt(HD)

    nc = bacc.Bacc("TRN2", target_bir_lowering=False, debug=False,
                   num_devices=num_devices)

    xT_d = nc.dram_tensor("xT", [EC * 128, T], bf16, kind="ExternalInput")
    wq_d = nc.dram_tensor("wq", [D // 128, 128, EC * 128], bf16,
                          kind="ExternalInput")
    wk_d = nc.dram_tensor("wk", [D // 128, 128, EC * 128], bf16,
                          kind="ExternalInput")
    wv_d = nc.dram_tensor("wv", [NVS, 128, EC * VN], bf16,
                          kind="ExternalInput")
    wp_d = nc.dram_tensor("wp", [D, NOD], bf16, kind="ExternalInput")
    tri_d = nc.dram_tensor("tri", [128, 128], bf16, kind="ExternalInput")
    ones_d = nc.dram_tensor("ones", [128, 128], bf16, kind="ExternalInput")
    y_d = nc.dram_tensor("y", [T, NOD], f32, kind="ExternalOutput")

    with tile.TileContext(nc) as tc:
        # ---- pools (manual lifetime management, dual-sided allocator) ----
        persist = tc.alloc_tile_pool(name="persist", bufs=1)            # L
        wv_pool = tc.alloc_tile_pool(name="wvp", bufs=1)                # L
        xthi_pool = tc.alloc_tile_pool(name="xthi", bufs=1)             # L
        xtlo_pool = tc.alloc_tile_pool(name="xtlo", bufs=1)             # L
        wcol_pool = tc.alloc_tile_pool(name="wcola", bufs=3)            # L
        qk_psum = tc.alloc_tile_pool(name="qk_psum", bufs=8, space="PSUM")

        # persistent tiles: q^T/k^T in [d, t] layout (head h = 128-row
        # chunk h), v in natural [t, d] layout ([t%128, tt*D + d]).
        qt_all = persist.tile([128, NH * T], bf16)
        kt_all = persist.tile([128, NH * T], bf16)
        v_all = persist.tile([128, TC * D], bf16)
        ones_sb = persist.tile([128, 128], bf16)
        tri_sb = persist.tile([128, 128], bf16)
        scr = persist.tile([1, 1], f32)

        # ---------------- phase A+B: q/k/v projections ----------------
        xt_lo = xtlo_pool.tile([128, EC * TLO], bf16)
        xt_hi = xthi_pool.tile([128, EC * (T - TLO)], bf16)

        def xt_tg(e, tg):
            # moving slice for q-group tg (TG=512) of e-chunk e
            off = tg * TG
            if off + TG <= TLO:
                return xt_lo[:, e * TLO + off:e * TLO + off + TG]
            off -= TLO
            return xt_hi[:, e * (T - TLO) + off:e * (T - TLO) + off + TG]

        def xt_tt(e, tt):
            # stationary slice for t-tile tt of e-chunk e
            off = tt * 128
            if off + 128 <= TLO:
                return xt_lo[:, e * TLO + off:e * TLO + off + 128]
            off -= TLO
            return xt_hi[:, e * (T - TLO) + off:e * (T - TLO) + off + 128]

        dsts = (qt_all, kt_all)
        wds = (wq_d, wk_d)
        wpairs = [(w_i, dc) for w_i in range(2) for dc in range(D // 128)]

        def load_wcol(w_i, dc, split=False):
            wcol = wcol_pool.tile([128, EC * 128], bf16, tag="wcol",
                                  name=f"wcol_{w_i}_{dc}")
            if split:
                # first two e-slices land first so the first LDWEIGHTS
                # fires after 64KB instead of 512KB
                nc.sync.dma_start(wcol[:, 0:256], wds[w_i][dc][:, 0:256])
                nc.sync.dma_start(wcol[:, 256:], wds[w_i][dc][:, 256:])
            else:
                nc.sync.dma_start(wcol[:], wds[w_i][dc])
            return wcol

        # Critical-path DMA order: sync queue = wq0a, wk0a, wq0b, wk0b,
        # then its share of x^T, then the next wcols; the other x^T
        # chunks ride scalar+gpsimd round-robin.  The first two waves
        # consume e-chunks in ARRIVAL order (PSUM accumulation is
        # commutative) so no single late chunk stalls the PE.
        wcol_q = [load_wcol(*wpairs[0], split=True),
                  load_wcol(*wpairs[1], split=True)]
        for e in range(EC):
            for (dst, lo) in ((xt_lo[:, e * TLO:(e + 1) * TLO], 0),
                              (xt_hi[:, e * (T - TLO):(e + 1) * (T - TLO)],
                               TLO)):
                src = xT_d[e * 128:(e + 1) * 128, lo:lo + (TLO if lo == 0
                                                           else T - TLO)]
                m = e % 3
                if m == 0:
                    nc.gpsimd.dma_start(out=dst, in_=src)
                elif m == 1:
                    nc.scalar.dma_start(dst, src)
                else:
                    nc.sync.dma_start(dst, src)
        wcol_q.append(load_wcol(*wpairs[2], split=True))
        wcol_q.append(load_wcol(*wpairs[3], split=True))
        # ones/tri + wv weights on the scalar queue (first needed at V /
        # attention, long after its x^T share drains); dummy exp pulls
        # the ~2.7us ACT table load off the attention critical path.
        nc.scalar.dma_start(ones_sb[:], ones_d[:])
        nc.scalar.dma_start(tri_sb[:], tri_d[:])
        nc.scalar.activation(scr[:], ones_sb[0:1, 0:1], AF.Exp, scale=1.0)
        wvgs = []
        for dg in range(NVS):
            wvg = wv_pool.tile([128, EC * VN], bf16, tag=f"wvg{dg}",
                               name=f"wvg_{dg}")
            nc.scalar.dma_start(wvg[:], wv_d[dg])
            wvgs.append(wvg)

        # arrival rank of e-chunks: queue q's k-th chunk lands ~(k+1)
        # chunk-times in; the sync queue is ~2 chunk-times behind (1MB of
        # wcol halves ahead of it).
        def arrival(e):
            q, k = e % 3, e // 3
            return (k + 1 + (2 if q == 2 else 0), e)

        e_order = sorted(range(EC), key=arrival)

        # Q^T / K^T: one (weight, d-chunk) per wave of 4 PSUM banks,
        # e-major inside the wave; 8 banks = two waves in flight.
        # The first TWO waves are e-interleaved so each arriving xT
        # chunk feeds 8 matmuls, matching the chunk DMA arrival rate.
        ngrp = T // TG

        def qk_wave_tiles(w_i, dc):
            return [qk_psum.tile([128, TG], f32, tag="abps",
                                 name=f"abps_{w_i}_{dc}_{tg}")
                    for tg in range(ngrp)]

        def qk_wave_mms(pss, wcol, e):
            for tg in range(ngrp):
                nc.tensor.matmul(
                    pss[tg][:],
                    wcol[:, e * 128:(e + 1) * 128],
                    xt_tg(e, tg),
                    start=(e == 0), stop=(e == EC - 1),
                )

        def qk_wave_copies(pss, w_i, dc):
            for tg in range(ngrp):
                nc.scalar.copy(
                    dsts[w_i][:, dc * T + tg * TG:dc * T + (tg + 1) * TG],
                    pss[tg][:])

        pss0 = qk_wave_tiles(*wpairs[0])
        pss1 = qk_wave_tiles(*wpairs[1])
        wcol0, wcol1 = wcol_q.pop(0), wcol_q.pop(0)
        for i, e in enumerate(e_order):
            st, sp = i == 0, i == EC - 1
            for tg in range(ngrp):
                for (pss, wcol) in ((pss0, wcol0), (pss1, wcol1)):
                    nc.tensor.matmul(
                        pss[tg][:], wcol[:, e * 128:(e + 1) * 128],
                        xt_tg(e, tg), start=st, stop=sp)
        qk_wave_copies(pss0, *wpairs[0])
        qk_wave_copies(pss1, *wpairs[1])
        for wi in range(2, len(wpairs)):
            w_i, dc = wpairs[wi]
            wcol = wcol_q.pop(0)
            if wi + 2 < len(wpairs):
                wcol_q.append(load_wcol(*wpairs[wi + 2]))
            pss = qk_wave_tiles(w_i, dc)
            for e in range(EC):
                qk_wave_mms(pss, wcol, e)
            qk_wave_copies(pss, w_i, dc)

        # V (natural [t, d] layout, full 512 moving width) for the LOW
        # t-tiles only; the high t-tiles are deferred into the attention
        # overlap below.  ACT is idle here so copies alternate ACT/DVE.
        for tt in range(TTLO):
            for dg in range(NVS):
                ps = qk_psum.tile([128, VN], f32, tag="abps",
                                  name=f"vps_{tt}_{dg}")
                for e in range(EC):
                    nc.tensor.matmul(
                        ps[:], xt_tt(e, tt),
                        wvgs[dg][:, e * VN:(e + 1) * VN],
                        start=(e == 0), stop=(e == EC - 1),
                    )
                dst = v_all[:, tt * D + dg * VN:tt * D + (dg + 1) * VN]
                if (tt * NVS + dg) % 2:
                    nc.vector.tensor_copy(dst, ps[:])
                else:
                    nc.scalar.copy(dst, ps[:])

        # ---------------- overlap window: free lo-half, open attention ----
        wcol_pool.release()
        xtlo_pool.release()
        qk_psum.release()

        v_psum = tc.alloc_tile_pool(name="v_psum", bufs=2, space="PSUM")
        s_psum = tc.alloc_tile_pool(name="s_psum", bufs=2, space="PSUM",
                                    side="right")
        a_psum = tc.alloc_tile_pool(name="a_psum", bufs=1, space="PSUM",
                                    side="right")
        d_psum = tc.alloc_tile_pool(name="d_psum", bufs=1, space="PSUM",
                                    side="right")
        cd1 = tc.alloc_tile_pool(name="cd1", bufs=1, side="right")
        pt_pool = tc.alloc_tile_pool(name="pt", bufs=3, side="right")
        sm_pool = tc.alloc_tile_pool(name="sm", bufs=2, side="right")

        # atn in qg-major layout: qg0/1 here, qg2/3 allocated post-overlap
        atn01 = cd1.tile([128, 2 * NH * TG], bf16)
        # Wp og-halves: og0/1 DMA can start as soon as cd1 exists (its
        # address range doesn't overlap the still-live xt_hi/wv tiles)
        wp_sb_lo = cd1.tile([128, NH * 2 * 512], bf16)
        nc.scalar.dma_start(
            wp_sb_lo.rearrange("p (hc og o) -> p hc og o", hc=NH, og=2),
            wp_d.rearrange("(hc p) (og o) -> p hc og o", p=128, o=512)
            [:, :, 0:2, :],
        )
        dsum_t = d_psum.tile([128, TG], f32)

        atn_hold = [atn01, None]   # atn23 filled in post-overlap

        def atn_slice(qg, h, lo, wdt):
            tile_ = atn_hold[qg // 2]
            base = (qg % 2) * NH * TG + h * TG + lo
            return tile_[:, base:base + wdt]

        # Deferred-V filler: emits V matmuls for t-tiles TTLO..TC-1 in
        # ~cycle-budgeted chunks between attention pieces.
        class VFiller:
            def __init__(self):
                self.items = [(tt, dg) for tt in range(TTLO, TC)
                              for dg in range(NVS)]
                self.idx = 0
                self.e = 0
                self.ps = None

            def done(self):
                return self.idx >= len(self.items)

            def take(self, ncy):
                n = (ncy + VN - 1) // VN
                while n > 0 and not self.done():
                    tt, dg = self.items[self.idx]
                    if self.ps is None:
                        self.ps = v_psum.tile([128, VN], f32, tag="vtps",
                                              name=f"vtail_{tt}_{dg}")
                        self.e = 0
                    e = self.e
                    nc.tensor.matmul(
                        self.ps[:], xt_tt(e, tt),
                        wvgs[dg][:, e * VN:(e + 1) * VN],
                        start=(e == 0), stop=(e == EC - 1),
                    )
                    self.e += 1
                    n -= 1
                    if self.e == EC:
                        dst = v_all[:, tt * D + dg * VN:
                                    tt * D + (dg + 1) * VN]
                        nc.vector.tensor_copy(dst, self.ps[:])
                        self.ps = None
                        self.idx += 1

            def drain(self):
                self.take(1 << 30)

        filler = VFiller()

        def emit_cgroup(qg, h, between=None):
            """Emit one (q-group, head) attention block.  `between(p)` is
            called after each pair's exp emission — PE filler goes there
            so the exp latency is always covered."""
            qbase = qg * TG
            npairs = 2 * (qg + 1)
            nk = 4 * (qg + 1)
            kc0 = qg * 4           # first diagonal k-chunk

            def pair_desc(p):
                # [(kc, soff, width, qoff)], exp width
                if p == npairs - 2:
                    return [(kc0, 0, 512, 0),
                            (kc0 + 1, 512, 384, 128)], 896
                if p == npairs - 1:
                    return [(kc0 + 2, 0, 256, 256),
                            (kc0 + 3, 256, 128, 384)], 384
                return [(2 * p, 0, 512, 0),
                        (2 * p + 1, 512, 512, 0)], 1024

            pts = [None] * npairs
            p_sum = pt_pool.tile([128, TG], bf16, tag="psacc",
                                 bufs=2, name=f"psacc_{qg}_{h}")

            def emit_av(p):
                parts, _ = pair_desc(p)
                p_t = pts[p]
                for (kc, soff, w, qoff) in parts:
                    nc.tensor.matmul(
                        atn_ps[:, qoff:qoff + w],
                        v_all[:, kc * D + h * HD:kc * D + (h + 1) * HD],
                        p_t[:, soff:soff + w],
                        start=(kc == 0), stop=(kc == nk - 1),
                    )
                for (kc, soff, w, qoff) in parts:
                    if kc == 0:
                        nc.vector.tensor_copy(p_sum[:], p_t[:, 0:TG])
                    else:
                        nc.vector.tensor_add(
                            p_sum[:, qoff:qoff + w],
                            p_sum[:, qoff:qoff + w],
                            p_t[:, soff:soff + w])

            atn_ps = a_psum.tile([128, TG], f32, tag="atn",
                                 name=f"atn_{qg}_{h}")
            for p in range(npairs):
                parts, expw = pair_desc(p)
                s_pair = s_psum.tile([128, 2 * TG], f32, tag="sp",
                                     name=f"sp_{qg}_{h}_{p}")
                packed = p == npairs - 1
                for pi, (kc, soff, w, qoff) in enumerate(parts):
                    nc.tensor.matmul(
                        s_pair[:, soff:soff + w],
                        kt_all[:, h * T + kc * 128:h * T + (kc + 1) * 128],
                        qt_all[:, h * T + qbase + qoff:h * T + qbase + 512],
                        start=(not packed or pi == 0),
                        stop=(not packed or pi == len(parts) - 1),
                    )
                p_t = pt_pool.tile([128, 2 * TG], bf16, tag="pt",
                                   name=f"pt_{qg}_{h}_{p}")
                pts[p] = p_t
                nc.scalar.activation(p_t[:, 0:expw], s_pair[:, 0:expw],
                                     AF.Exp, scale=scale)
                if p >= npairs - 2:
                    for (kc, soff, w, qoff) in parts:
                        nc.vector.tensor_mul(
                            p_t[:, soff:soff + 128],
                            p_t[:, soff:soff + 128],
                            tri_sb[:])
                if between is not None:
                    between(p)
                if p > 0:
                    emit_av(p - 1)
            emit_av(npairs - 1)
            slot = qg * NH + h
            atn_u = sm_pool.tile([128, TG], bf16, tag="atnu",
                                 name=f"atnu_{slot}")
            nc.scalar.copy(atn_u[:], atn_ps[:])

            def finalize():
                # ones[128,128]-matmul broadcasts the column-sums of
                # p_sum to every partition; single-pass DVE reciprocal;
                # gpsimd multiply into the atn tile.  Deferred past the
                # caller's interleaved PE work so the DVE p_sum chain
                # drains off the PE critical path.
                nc.tensor.matmul(dsum_t[:], ones_sb[:], p_sum[:],
                                 start=True, stop=True)
                recipB = sm_pool.tile([128, TG], f32, tag="rB",
                                      name=f"rB_{slot}")
                nc.vector.reciprocal_approx_fast(out=recipB[:],
                                                 in_=dsum_t[:])
                nc.gpsimd.tensor_mul(
                    atn_slice(qg, h, 0, TG), atn_u[:], recipB[:])

            return finalize

        # Overlap: qg=0 and qg=1 attention with deferred-V as PE filler.
        # Consumes 160 of the 256 deferred-V matmuls between pairs; the
        # remainder drains as one solid PE block before qg=2.
        for qg in range(2):
            for h in range(NH):
                fin = emit_cgroup(qg, h, between=lambda p: filler.take(1024))
                filler.take(2048)
                fin()
        filler.drain()

        # ---------------- phase C+D proper: qg2/3 + out-projection ----
        xthi_pool.release()
        wv_pool.release()
        v_psum.release()
        y_psum = tc.alloc_tile_pool(name="y_psum", bufs=2, space="PSUM")
        cd2 = tc.alloc_tile_pool(name="cd2", bufs=1, side="right")
        yst_pool = tc.alloc_tile_pool(name="yst", bufs=2, side="right")
        atn23 = cd2.tile([128, 2 * NH * TG], bf16)
        atn_hold[1] = atn23
        wp_sb_hi = cd2.tile([128, NH * 2 * 512], bf16)
        nc.sync.dma_start(
            wp_sb_hi.rearrange("p (hc og o) -> p hc og o", hc=NH, og=2),
            wp_d.rearrange("(hc p) (og o) -> p hc og o", p=128, o=512)
            [:, :, 2:4, :],
        )

        ndb = [0]

        def emit_dblock(tt, og):
            qg, ttq = tt // 4, tt % 4
            wp_t = wp_sb_lo if og < 2 else wp_sb_hi
            ogl = og % 2
            ps = y_psum.tile([128, 512], f32, tag="yps",
                             name=f"yps_{tt}_{og}")
            for hc in range(NH):
                nc.tensor.matmul(
                    ps[:],
                    atn_slice(qg, hc, ttq * 128, 128),
                    wp_t[:, (hc * 2 + ogl) * 512:(hc * 2 + ogl + 1) * 512],
                    start=(hc == 0), stop=(hc == NH - 1),
                )
            yst = yst_pool.tile([128, 512], f32, tag="yst",
                                name=f"yst_{tt}_{og}")
            if ndb[0] % 2:
                nc.vector.tensor_copy(yst[:], ps[:])
            else:
                nc.scalar.copy(yst[:], ps[:])
            dst = y_d[tt * 128:(tt + 1) * 128, og * 512:(og + 1) * 512]
            m = ndb[0] % 3
            ndb[0] += 1
            if m == 0:
                nc.sync.dma_start(dst, yst[:])
            elif m == 1:
                nc.scalar.dma_start(dst, yst[:])
            else:
                nc.gpsimd.dma_start(out=dst, in_=yst[:])

        # dblock queue: og-major within each qg batch so the Wp og-halves
        # arrive in drain order
        dq = deque()
        for qg in range(2):
            for og in range(ODG):
                for tt in range(qg * 4, qg * 4 + 4):
                    dq.append((tt, og))

        def drain(n):
            for _ in range(n):
                if dq:
                    emit_dblock(*dq.popleft())

        # qg2: 3 dblocks inside each cgroup
        for h in range(NH):
            fin = emit_cgroup(
                2, h, between=lambda p: drain(1) if p in (1, 3, 5) else None)
            fin()
        for og in range(ODG):
            for tt in range(8, 12):
                dq.append((tt, og))
        # qg3: 3 dblocks inside each cgroup
        for h in range(NH):
            fin = emit_cgroup(
                3, h, between=lambda p: drain(1) if p in (1, 3, 5) else None)
            fin()
        for og in range(ODG):
            for tt in range(12, 16):
                dq.append((tt, og))
        while dq:
            emit_dblock(*dq.popleft())

        # release remaining pools (LIFO per side)
        y_psum.release()
        d_psum.release()
        a_psum.release()
        s_psum.release()
        yst_pool.release()
        cd2.release()
        sm_pool.release()
        pt_pool.release()
        cd1.release()
        persist.release()

    nc.compile()
    return nc


def _augment(mat, bias_row, pad_to):
    """Append [bias_row; zeros] below mat so it has pad_to rows."""
    extra = np.zeros((pad_to - mat.shape[0], mat.shape[1]), np.float32)
    extra[0] = bias_row
    return np.concatenate([mat, extra], axis=0)


def _swizzle_qk(w, EC):
    """[EC*128, D] -> [D//128, 128, EC*128]: per-wave slice partition-major
    so its DMA moves in 4KB packets."""
    D = w.shape[1]
    return np.ascontiguousarray(
        w.reshape(EC, 128, D // 128, 128).transpose(2, 1, 0, 3)
        .reshape(D // 128, 128, EC * 128).astype(BF))


def _swizzle_v(w, EC, VN=512):
    """[EC*128, D] -> [D//VN, 128, EC*VN] partition-major."""
    D = w.shape[1]
    return np.ascontiguousarray(
        w.reshape(EC, 128, D // VN, VN).transpose(2, 1, 0, 3)
        .reshape(D // VN, 128, EC * VN).astype(BF))


_NC_CACHE = {}


def _get_nc(bias):
    if bias not in _NC_CACHE:
        _NC_CACHE[bias] = build_nc(bias=bias)
    return _NC_CACHE[bias]


def kernel(x, Wq, bq, Wk, bk, Wv, bv, Wp, bp):
    global LAST_RESULT
    x = np.ascontiguousarray(np.asarray(x, np.float32))
    Wq, bq = np.asarray(Wq, np.float32), np.asarray(bq, np.float32)
    Wk, bk = np.asarray(Wk, np.float32), np.asarray(bk, np.float32)
    Wv, bv = np.asarray(Wv, np.float32), np.asarray(bv, np.float32)
    Wp, bp = np.asarray(Wp, np.float32), np.asarray(bp, np.float32)

    B, T, C = x.shape
    assert (B, T, C) == (4, 2048, 2048), (B, T, C)
    D = 1024  # head-group width: 8 heads per core
    bias = bool(np.any(bq) or np.any(bk) or np.any(bv))
    nc = _get_nc(bias)

    kk = np.arange(128)[:, None]
    qq = np.arange(128)[None, :]
    tri = (kk <= qq).astype(BF)
    ones = np.ones((128, 128), BF)
    Ep = C + 128 if bias else C

    in_maps = []
    for c in range(N_CORES):
        b, g = c // 2, c % 2
        xt = x[b].T
        wq_g = Wq[:, g * D:(g + 1) * D]
        wk_g = Wk[:, g * D:(g + 1) * D]
        wv_g = Wv[:, g * D:(g + 1) * D]
        if bias:
            xt = _augment(xt, np.ones(T, np.float32), Ep)
            wq_g = _augment(wq_g, bq[g * D:(g + 1) * D], Ep)
            wk_g = _augment(wk_g, bk[g * D:(g + 1) * D], Ep)
            wv_g = _augment(wv_g, bv[g * D:(g + 1) * D], Ep)
        EC = Ep // 128
        in_maps.append({
            "xT": np.ascontiguousarray(xt.astype(BF)),
            "wq": _swizzle_qk(wq_g, EC),
            "wk": _swizzle_qk(wk_g, EC),
            "wv": _swizzle_v(wv_g, EC),
            "wp": np.ascontiguousarray(Wp[g * D:(g + 1) * D, :].astype(BF)),
            "tri": tri,
            "ones": ones,
        })

    trace = bool(os.environ.get("MHA_TRACE"))
    res = run_bass_kernel_spmd(nc, in_maps, core_ids=list(range(N_CORES)),
                               trace=trace)
    LAST_RESULT = res

    out = np.empty((B, T, C), np.float32)
    for b in range(B):
        out[b] = res.results[2 * b]["y"] + res.results[2 * b + 1]["y"]
    out += bp[None, None, :]
    return out


# revision 14
# speedup vs baseline: 1.0139x; 1.0139x over previous
"""TRN2 Bass kernel for 16-head causal MHA (B=4, T=2048, C=2048), fp32 in/out.

Sharding: 8 cores = 4 batches x 2 head-groups (8 heads each).  Each core
computes q/k/v projections for its head group on its batch (tensor-parallel
column split of Wq/Wk/Wv), causal attention in the S^T layout, and a partial
output projection with the row slice of Wp.  The two head-group partials per
batch are summed on the host, plus the output bias.

Design (v7, bf16 datapath):
- All operand data is bf16; every matmul accumulates in fp32 PSUM.
- Startup: DMA emission is ordered by first consumption.  The sync queue
  carries only the Q/K weight columns (first two waves' stationaries split
  so the first LDWEIGHTS fires after 64KB); x^T chunks ride the scalar +
  gpsimd queues in consumption order; ones/tri/wvg/wp follow later.
- Q^T/K^T via e-interleaved wave pairs (8 PSUM banks) as before; V is
  computed for t-tiles 0..7 in the projection phase, while t-tiles 8..15
  are DEFERRED and emitted as PE filler inside the qg=0/qg=1 attention
  stretch (which is otherwise exp-latency bound).  x^T is split into two
  tiles (t<1024 / t>=1024) so only the high half (+wv weights) stays
  resident during that overlap window.
- Attention (S^T layout, multiplicative causal mask on diagonal blocks,
  packed variable-width score/attn matmuls, ones-matmul softmax
  denominator broadcast, single-pass DVE reciprocal, gpsimd normalize)
  is unchanged except: atn lives in two qg-major tiles (qg0/1, qg2/3),
  and out-projection D-blocks are interleaved WITHIN qg2/qg3 cgroups
  (between score pairs) so exp latency never stalls the PE.
- Output projection column groups drain og-major so the og-sliced Wp DMA
  (issued as SBUF frees up after the overlap window) arrives just in time;
  y tiles DMA out round-robin across the sync/scalar/gpsimd queues.
"""
import math
import os
from collections import deque

import ml_dtypes
import numpy as np

import concourse.bass as bass
import concourse.tile as tile
from concourse import bacc, mybir
from concourse.bass_utils import run_bass_kernel_spmd

f32 = mybir.dt.float32
bf16 = mybir.dt.bfloat16
AF = mybir.ActivationFunctionType
BF = ml_dtypes.bfloat16

N_CORES = 8
HD = 128                      # head dim

# results of the last run_bass_kernel_spmd call (for test harness profiling)
LAST_RESULT = None


def build_nc(T=2048, E=2048, D=1024, NOD=2048, TG=512, bias=False,
             num_devices=N_CORES):
    """Build + compile the per-core Bass program."""
    NH = D // HD              # heads per core
    EC = E // 128 + (1 if bias else 0)
    TC = T // 128             # 128-row tiles along T
    TGC = T // TG             # q-groups
    ODG = NOD // 512          # out-proj column groups
    VN = 512                  # v-projection moving width
    NVS = D // VN
    TLO = T // 2              # t-split point: xt_lo covers t<TLO
    TTLO = TLO // 128         # t-tiles in the low half (V in phase B)
    scale = 1.0 / math.sqrt(HD)

    nc = bacc.Bacc("TRN2", target_bir_lowering=False, debug=False,
                   num_devices=num_devices)

    xT_d = nc.dram_tensor("xT", [EC * 128, T], bf16, kind="ExternalInput")
    wq_d = nc.dram_tensor("wq", [D // 128, 128, EC * 128], bf16,
                          kind="ExternalInput")
    wk_d = nc.dram_tensor("wk", [D // 128, 128, EC * 128], bf16,
                          kind="ExternalInput")
    wv_d = nc.dram_tensor("wv", [NVS, 128, EC * VN], bf16,
                          kind="ExternalInput")
    wp_d = nc.dram_tensor("wp", [D, NOD], bf16, kind="ExternalInput")
    tri_d = nc.dram_tensor("tri", [128, 128], bf16, kind="ExternalInput")
    ones_d = nc.dram_tensor("ones", [128, 128], bf16, kind="ExternalInput")
    y_d = nc.dram_tensor("y", [T, NOD], f32, kind="ExternalOutput")

    with tile.TileContext(nc) as tc:
        # ---- pools (manual lifetime management, dual-sided allocator) ----
        persist = tc.alloc_tile_pool(name="persist", bufs=1)            # L
        wv_pool = tc.alloc_tile_pool(name="wvp", bufs=1)                # L
        xthi_pool = tc.alloc_tile_pool(name="xthi", bufs=1)             # L
        xtlo_pool = tc.alloc_tile_pool(name="xtlo", bufs=1)             # L
        wcol_pool = tc.alloc_tile_pool(name="wcola", bufs=3)            # L
        qk_psum = tc.alloc_tile_pool(name="qk_psum", bufs=8, space="PSUM")

        # persistent tiles: q^T/k^T in [d, t] layout (head h = 128-row
        # chunk h), v in natural [t, d] layout ([t%128, tt*D + d]).
        qt_all = persist.tile([128, NH * T], bf16)
        kt_all = persist.tile([128, NH * T], bf16)
        v_all = persist.tile([128, TC * D], bf16)
        ones_sb = persist.tile([128, 128], bf16)
        tri_sb = persist.tile([128, 128], bf16)
        scr = persist.tile([1, 1], f32)

        # ---------------- phase A+B: q/k/v projections ----------------
        xt_lo = xtlo_pool.tile([128, EC * TLO], bf16)
        xt_hi = xthi_pool.tile([128, EC * (T - TLO)], bf16)

        def xt_tg(e, tg):
            # moving slice for q-group tg (TG=512) of e-chunk e
            off = tg * TG
            if off + TG <= TLO:
                return xt_lo[:, e * TLO + off:e * TLO + off + TG]
            off -= TLO
            return xt_hi[:, e * (T - TLO) + off:e * (T - TLO) + off + TG]

        def xt_tt(e, tt):
            # stationary slice for t-tile tt of e-chunk e
            off = tt * 128
            if off + 128 <= TLO:
                return xt_lo[:, e * TLO + off:e * TLO + off + 128]
            off -= TLO
            return xt_hi[:, e * (T - TLO) + off:e * (T - TLO) + off + 128]

        dsts = (qt_all, kt_all)
        wds = (wq_d, wk_d)
        wpairs = [(w_i, dc) for w_i in range(2) for dc in range(D // 128)]

        def load_wcol(w_i, dc, split=False):
            wcol = wcol_pool.tile([128, EC * 128], bf16, tag="wcol",
                                  name=f"wcol_{w_i}_{dc}")
            if split:
                # first two e-slices land first so the first LDWEIGHTS
                # fires after 64KB instead of 512KB
                nc.sync.dma_start(wcol[:, 0:256], wds[w_i][dc][:, 0:256])
                nc.sync.dma_start(wcol[:, 256:], wds[w_i][dc][:, 256:])
            else:
                nc.sync.dma_start(wcol[:], wds[w_i][dc])
            return wcol

        # Critical-path DMA order: sync queue = wq0a, wk0a, wq0b, wk0b,
        # then its share of x^T, then the next wcols; the other x^T
        # chunks ride scalar+gpsimd round-robin.  The first two waves
        # consume e-chunks in ARRIVAL order (PSUM accumulation is
        # commutative) so no single late chunk stalls the PE.
        # x^T queue split: the sync queue is busy with 2MB of wcols during
        # the slow HBM ramp, so it only gets 2 late chunks; gpsimd/scalar
        # alternate the rest.
        y_set = sorted({EC // 3, 2 * EC // 3 + 1})
        rest = [e for e in range(EC) if e not in y_set]
        g_list = rest[0::2]
        s_list = rest[1::2]
        wcol_q = [load_wcol(*wpairs[0], split=True),
                  load_wcol(*wpairs[1], split=True)]
        for e in range(EC):
            for (dst, lo) in ((xt_lo[:, e * TLO:(e + 1) * TLO], 0),
                              (xt_hi[:, e * (T - TLO):(e + 1) * (T - TLO)],
                               TLO)):
                src = xT_d[e * 128:(e + 1) * 128, lo:lo + (TLO if lo == 0
                                                           else T - TLO)]
                if e in g_list:
                    nc.gpsimd.dma_start(out=dst, in_=src)
                elif e in s_list:
                    nc.scalar.dma_start(dst, src)
                else:
                    nc.sync.dma_start(dst, src)
        wcol_q.append(load_wcol(*wpairs[2], split=True))
        wcol_q.append(load_wcol(*wpairs[3], split=True))
        # ones/tri + wv weights on the scalar queue (first needed at V /
        # attention, long after its x^T share drains); dummy exp pulls
        # the ~2.7us ACT table load off the attention critical path.
        nc.scalar.dma_start(ones_sb[:], ones_d[:])
        nc.scalar.dma_start(tri_sb[:], tri_d[:])
        nc.scalar.activation(scr[:], ones_sb[0:1, 0:1], AF.Exp, scale=1.0)
        wvgs = []
        for dg in range(NVS):
            wvg = wv_pool.tile([128, EC * VN], bf16, tag=f"wvg{dg}",
                               name=f"wvg_{dg}")
            nc.scalar.dma_start(wvg[:], wv_d[dg])
            wvgs.append(wvg)

        # arrival rank of e-chunks: queue q's k-th chunk lands ~(k+1)
        # chunk-times in; the sync queue's are ~5 chunk-times apart
        # (behind 2MB of wcol halves).
        e_order = [e for _, e in sorted(
            [(k + 1.0, e) for k, e in enumerate(g_list)]
            + [(k + 1.01, e) for k, e in enumerate(s_list)]
            + [(6.5 + 5.0 * k, e) for k, e in enumerate(y_set)])]

        # Q^T / K^T: one (weight, d-chunk) per wave of 4 PSUM banks,
        # e-major inside the wave; 8 banks = two waves in flight.
        # The first TWO waves are e-interleaved so each arriving xT
        # chunk feeds 8 matmuls, matching the chunk DMA arrival rate.
        ngrp = T // TG

        def qk_wave_tiles(w_i, dc):
            return [qk_psum.tile([128, TG], f32, tag="abps",
                                 name=f"abps_{w_i}_{dc}_{tg}")
                    for tg in range(ngrp)]

        def qk_wave_mms(pss, wcol, e):
            for tg in range(ngrp):
                nc.tensor.matmul(
                    pss[tg][:],
                    wcol[:, e * 128:(e + 1) * 128],
                    xt_tg(e, tg),
                    start=(e == 0), stop=(e == EC - 1),
                )

        def qk_wave_copies(pss, w_i, dc):
            for tg in range(ngrp):
                nc.scalar.copy(
                    dsts[w_i][:, dc * T + tg * TG:dc * T + (tg + 1) * TG],
                    pss[tg][:])

        pss0 = qk_wave_tiles(*wpairs[0])
        pss1 = qk_wave_tiles(*wpairs[1])
        wcol0, wcol1 = wcol_q.pop(0), wcol_q.pop(0)
        for i, e in enumerate(e_order):
            st, sp = i == 0, i == EC - 1
            for tg in range(ngrp):
                for (pss, wcol) in ((pss0, wcol0), (pss1, wcol1)):
                    nc.tensor.matmul(
                        pss[tg][:], wcol[:, e * 128:(e + 1) * 128],
                        xt_tg(e, tg), start=st, stop=sp)
        qk_wave_copies(pss0, *wpairs[0])
        qk_wave_copies(pss1, *wpairs[1])
        for wi in range(2, len(wpairs)):
            w_i, dc = wpairs[wi]
            wcol = wcol_q.pop(0)
            if wi + 2 < len(wpairs):
                wcol_q.append(load_wcol(*wpairs[wi + 2]))
            pss = qk_wave_tiles(w_i, dc)
            for e in range(EC):
                qk_wave_mms(pss, wcol, e)
            qk_wave_copies(pss, w_i, dc)

        # V (natural [t, d] layout, full 512 moving width) for the LOW
        # t-tiles only; the high t-tiles are deferred into the attention
        # overlap below.  ACT is idle here so copies alternate ACT/DVE.
        for tt in range(TTLO):
            for dg in range(NVS):
                ps = qk_psum.tile([128, VN], f32, tag="abps",
                                  name=f"vps_{tt}_{dg}")
                for e in range(EC):
                    nc.tensor.matmul(
                        ps[:], xt_tt(e, tt),
                        wvgs[dg][:, e * VN:(e + 1) * VN],
                        start=(e == 0), stop=(e == EC - 1),
                    )
                dst = v_all[:, tt * D + dg * VN:tt * D + (dg + 1) * VN]
                if (tt * NVS + dg) % 2:
                    nc.vector.tensor_copy(dst, ps[:])
                else:
                    nc.scalar.copy(dst, ps[:])

        # ---------------- overlap window: free lo-half, open attention ----
        wcol_pool.release()
        xtlo_pool.release()
        qk_psum.release()

        v_psum = tc.alloc_tile_pool(name="v_psum", bufs=2, space="PSUM")
        s_psum = tc.alloc_tile_pool(name="s_psum", bufs=2, space="PSUM",
                                    side="right")
        a_psum = tc.alloc_tile_pool(name="a_psum", bufs=1, space="PSUM",
                                    side="right")
        d_psum = tc.alloc_tile_pool(name="d_psum", bufs=1, space="PSUM",
                                    side="right")
        cd1 = tc.alloc_tile_pool(name="cd1", bufs=1, side="right")
        pt_pool = tc.alloc_tile_pool(name="pt", bufs=3, side="right")
        sm_pool = tc.alloc_tile_pool(name="sm", bufs=2, side="right")

        # atn in qg-major layout: qg0/1 here, qg2/3 allocated post-overlap
        atn01 = cd1.tile([128, 2 * NH * TG], bf16)
        # Wp og-halves: og0/1 DMA can start as soon as cd1 exists (its
        # address range doesn't overlap the still-live xt_hi/wv tiles)
        wp_sb_lo = cd1.tile([128, NH * 2 * 512], bf16)
        nc.scalar.dma_start(
            wp_sb_lo.rearrange("p (hc og o) -> p hc og o", hc=NH, og=2),
            wp_d.rearrange("(hc p) (og o) -> p hc og o", p=128, o=512)
            [:, :, 0:2, :],
        )
        dsum_t = d_psum.tile([128, TG], f32)

        atn_hold = [atn01, None]   # atn23 filled in post-overlap

        def atn_slice(qg, h, lo, wdt):
            tile_ = atn_hold[qg // 2]
            base = (qg % 2) * NH * TG + h * TG + lo
            return tile_[:, base:base + wdt]

        # Deferred-V filler: emits V matmuls for t-tiles TTLO..TC-1 in
        # ~cycle-budgeted chunks between attention pieces.
        class VFiller:
            def __init__(self):
                self.items = [(tt, dg) for tt in range(TTLO, TC)
                              for dg in range(NVS)]
                self.idx = 0
                self.e = 0
                self.ps = None

            def done(self):
                return self.idx >= len(self.items)

            def take(self, ncy):
                n = (ncy + VN - 1) // VN
                while n > 0 and not self.done():
                    tt, dg = self.items[self.idx]
                    if self.ps is None:
                        self.ps = v_psum.tile([128, VN], f32, tag="vtps",
                                              name=f"vtail_{tt}_{dg}")
                        self.e = 0
                    e = self.e
                    nc.tensor.matmul(
                        self.ps[:], xt_tt(e, tt),
                        wvgs[dg][:, e * VN:(e + 1) * VN],
                        start=(e == 0), stop=(e == EC - 1),
                    )
                    self.e += 1
                    n -= 1
                    if self.e == EC:
                        dst = v_all[:, tt * D + dg * VN:
                                    tt * D + (dg + 1) * VN]
                        nc.vector.tensor_copy(dst, self.ps[:])
                        self.ps = None
                        self.idx += 1

            def drain(self):
                self.take(1 << 30)

        filler = VFiller()

        def emit_cgroup(qg, h, between=None):
            """Emit one (q-group, head) attention block.  `between(p)` is
            called after each pair's exp emission — PE filler goes there
            so the exp latency is always covered."""
            qbase = qg * TG
            npairs = 2 * (qg + 1)
            nk = 4 * (qg + 1)
            kc0 = qg * 4           # first diagonal k-chunk

            def pair_desc(p):
                # [(kc, soff, width, qoff)], exp width
                if p == npairs - 2:
                    return [(kc0, 0, 512, 0),
                            (kc0 + 1, 512, 384, 128)], 896
                if p == npairs - 1:
                    return [(kc0 + 2, 0, 256, 256),
                            (kc0 + 3, 256, 128, 384)], 384
                return [(2 * p, 0, 512, 0),
                        (2 * p + 1, 512, 512, 0)], 1024

            pts = [None] * npairs
            p_sum = pt_pool.tile([128, TG], bf16, tag="psacc",
                                 bufs=2, name=f"psacc_{qg}_{h}")

            def emit_av(p):
                parts, _ = pair_desc(p)
                p_t = pts[p]
                for (kc, soff, w, qoff) in parts:
                    nc.tensor.matmul(
                        atn_ps[:, qoff:qoff + w],
                        v_all[:, kc * D + h * HD:kc * D + (h + 1) * HD],
                        p_t[:, soff:soff + w],
                        start=(kc == 0), stop=(kc == nk - 1),
                    )
                for (kc, soff, w, qoff) in parts:
                    if kc == 0:
                        nc.vector.tensor_copy(p_sum[:], p_t[:, 0:TG])
                    else:
                        nc.vector.tensor_add(
                            p_sum[:, qoff:qoff + w],
                            p_sum[:, qoff:qoff + w],
                            p_t[:, soff:soff + w])

            atn_ps = a_psum.tile([128, TG], f32, tag="atn",
                                 name=f"atn_{qg}_{h}")
            for p in range(npairs):
                parts, expw = pair_desc(p)
                s_pair = s_psum.tile([128, 2 * TG], f32, tag="sp",
                                     name=f"sp_{qg}_{h}_{p}")
                packed = p == npairs - 1
                for pi, (kc, soff, w, qoff) in enumerate(parts):
                    nc.tensor.matmul(
                        s_pair[:, soff:soff + w],
                        kt_all[:, h * T + kc * 128:h * T + (kc + 1) * 128],
                        qt_all[:, h * T + qbase + qoff:h * T + qbase + 512],
                        start=(not packed or pi == 0),
                        stop=(not packed or pi == len(parts) - 1),
                    )
                p_t = pt_pool.tile([128, 2 * TG], bf16, tag="pt",
                                   name=f"pt_{qg}_{h}_{p}")
                pts[p] = p_t
                nc.scalar.activation(p_t[:, 0:expw], s_pair[:, 0:expw],
                                     AF.Exp, scale=scale)
                if p >= npairs - 2:
                    for (kc, soff, w, qoff) in parts:
                        nc.vector.tensor_mul(
                            p_t[:, soff:soff + 128],
                            p_t[:, soff:soff + 128],
                            tri_sb[:])
                if between is not None:
                    between(p)
                if p > 0:
                    emit_av(p - 1)
            emit_av(npairs - 1)
            slot = qg * NH + h
            atn_u = sm_pool.tile([128, TG], bf16, tag="atnu",
                                 name=f"atnu_{slot}")
            nc.scalar.copy(atn_u[:], atn_ps[:])

            def finalize():
                # ones[128,128]-matmul broadcasts the column-sums of
                # p_sum to every partition; single-pass DVE reciprocal;
                # gpsimd multiply into the atn tile.  Deferred past the
                # caller's interleaved PE work so the DVE p_sum chain
                # drains off the PE critical path.
                nc.tensor.matmul(dsum_t[:], ones_sb[:], p_sum[:],
                                 start=True, stop=True)
                recipB = sm_pool.tile([128, TG], f32, tag="rB",
                                      name=f"rB_{slot}")
                nc.vector.reciprocal_approx_fast(out=recipB[:],
                                                 in_=dsum_t[:])
                nc.gpsimd.tensor_mul(
                    atn_slice(qg, h, 0, TG), atn_u[:], recipB[:])

            return finalize

        # Overlap: qg=0 and qg=1 attention with deferred-V as PE filler.
        # Consumes 160 of the 256 deferred-V matmuls between pairs; the
        # remainder drains as one solid PE block before qg=2.
        for qg in range(2):
            for h in range(NH):
                fin = emit_cgroup(qg, h, between=lambda p: filler.take(1024))
                filler.take(2048)
                fin()
        filler.drain()

        # ---------------- phase C+D proper: qg2/3 + out-projection ----
        xthi_pool.release()
        wv_pool.release()
        v_psum.release()
        y_psum = tc.alloc_tile_pool(name="y_psum", bufs=2, space="PSUM")
        cd2 = tc.alloc_tile_pool(name="cd2", bufs=1, side="right")
        yst_pool = tc.alloc_tile_pool(name="yst", bufs=2, side="right")
        atn23 = cd2.tile([128, 2 * NH * TG], bf16)
        atn_hold[1] = atn23
        wp_sb_hi = cd2.tile([128, NH * 2 * 512], bf16)
        nc.sync.dma_start(
            wp_sb_hi.rearrange("p (hc og o) -> p hc og o", hc=NH, og=2),
            wp_d.rearrange("(hc p) (og o) -> p hc og o", p=128, o=512)
            [:, :, 2:4, :],
        )

        ndb = [0]

        def emit_dblock(tt, og):
            qg, ttq = tt // 4, tt % 4
            wp_t = wp_sb_lo if og < 2 else wp_sb_hi
            ogl = og % 2
            ps = y_psum.tile([128, 512], f32, tag="yps",
                             name=f"yps_{tt}_{og}")
            for hc in range(NH):
                nc.tensor.matmul(
                    ps[:],
                    atn_slice(qg, hc, ttq * 128, 128),
                    wp_t[:, (hc * 2 + ogl) * 512:(hc * 2 + ogl + 1) * 512],
                    start=(hc == 0), stop=(hc == NH - 1),
                )
            yst = yst_pool.tile([128, 512], f32, tag="yst",
                                name=f"yst_{tt}_{og}")
            if ndb[0] % 2:
                nc.vector.tensor_copy(yst[:], ps[:])
            else:
                nc.scalar.copy(yst[:], ps[:])
            dst = y_d[tt * 128:(tt + 1) * 128, og * 512:(og + 1) * 512]
            m = ndb[0] % 3
            ndb[0] += 1
            if m == 0:
                nc.sync.dma_start(dst, yst[:])
            elif m == 1:
                nc.scalar.dma_start(dst, yst[:])
            else:
                nc.gpsimd.dma_start(out=dst, in_=yst[:])

        # dblock queue: og-major within each qg batch so the Wp og-halves
        # arrive in drain order
        dq = deque()
        for qg in range(2):
            for og in range(ODG):
                for tt in range(qg * 4, qg * 4 + 4):
                    dq.append((tt, og))

        def drain(n):
            for _ in range(n):
                if dq:
                    emit_dblock(*dq.popleft())

        # qg2: 3 dblocks inside each cgroup
        for h in range(NH):
            fin = emit_cgroup(
                2, h, between=lambda p: drain(1) if p in (1, 3, 5) else None)
            fin()
        for og in range(ODG):
            for tt in range(8, 12):
                dq.append((tt, og))
        # qg3: 3 dblocks inside each cgroup
        for h in range(NH):
            fin = emit_cgroup(
                3, h, between=lambda p: drain(1) if p in (1, 3, 5) else None)
            fin()
        for og in range(ODG):
            for tt in range(12, 16):
                dq.append((tt, og))
        while dq:
            emit_dblock(*dq.popleft())

        # release remaining pools (LIFO per side)
        y_psum.release()
        d_psum.release()
        a_psum.release()
        s_psum.release()
        yst_pool.release()
        cd2.release()
        sm_pool.release()
        pt_pool.release()
        cd1.release()
        persist.release()

    nc.compile()
    return nc


def _augment(mat, bias_row, pad_to):
    """Append [bias_row; zeros] below mat so it has pad_to rows."""
    extra = np.zeros((pad_to - mat.shape[0], mat.shape[1]), np.float32)
    extra[0] = bias_row
    return np.concatenate([mat, extra], axis=0)


def _swizzle_qk(w, EC):
    """[EC*128, D] -> [D//128, 128, EC*128]: per-wave slice partition-major
    so its DMA moves in 4KB packets."""
    D = w.shape[1]
    return np.ascontiguousarray(
        w.reshape(EC, 128, D // 128, 128).transpose(2, 1, 0, 3)
        .reshape(D // 128, 128, EC * 128).astype(BF))


def _swizzle_v(w, EC, VN=512):
    """[EC*128, D] -> [D//VN, 128, EC*VN] partition-major."""
    D = w.shape[1]
    return np.ascontiguousarray(
        w.reshape(EC, 128, D // VN, VN).transpose(2, 1, 0, 3)
        .reshape(D // VN, 128, EC * VN).astype(BF))


_NC_CACHE = {}


def _get_nc(bias):
    if bias not in _NC_CACHE:
        _NC_CACHE[bias] = build_nc(bias=bias)
    return _NC_CACHE[bias]


def kernel(x, Wq, bq, Wk, bk, Wv, bv, Wp, bp):
    global LAST_RESULT
    x = np.ascontiguousarray(np.asarray(x, np.float32))
    Wq, bq = np.asarray(Wq, np.float32), np.asarray(bq, np.float32)
    Wk, bk = np.asarray(Wk, np.float32), np.asarray(bk, np.float32)
    Wv, bv = np.asarray(Wv, np.float32), np.asarray(bv, np.float32)
    Wp, bp = np.asarray(Wp, np.float32), np.asarray(bp, np.float32)

    B, T, C = x.shape
    assert (B, T, C) == (4, 2048, 2048), (B, T, C)
    D = 1024  # head-group width: 8 heads per core
    bias = bool(np.any(bq) or np.any(bk) or np.any(bv))
    nc = _get_nc(bias)

    kk = np.arange(128)[:, None]
    qq = np.arange(128)[None, :]
    tri = (kk <= qq).astype(BF)
    ones = np.ones((128, 128), BF)
    Ep = C + 128 if bias else C

    in_maps = []
    for c in range(N_CORES):
        b, g = c // 2, c % 2
        xt = x[b].T
        wq_g = Wq[:, g * D:(g + 1) * D]
        wk_g = Wk[:, g * D:(g + 1) * D]
        wv_g = Wv[:, g * D:(g + 1) * D]
        if bias:
            xt = _augment(xt, np.ones(T, np.float32), Ep)
            wq_g = _augment(wq_g, bq[g * D:(g + 1) * D], Ep)
            wk_g = _augment(wk_g, bk[g * D:(g + 1) * D], Ep)
            wv_g = _augment(wv_g, bv[g * D:(g + 1) * D], Ep)
        EC = Ep // 128
        in_maps.append({
            "xT": np.ascontiguousarray(xt.astype(BF)),
            "wq": _swizzle_qk(wq_g, EC),
            "wk": _swizzle_qk(wk_g, EC),
            "wv": _swizzle_v(wv_g, EC),
            "wp": np.ascontiguousarray(Wp[g * D:(g + 1) * D, :].astype(BF)),
            "tri": tri,
            "ones": ones,
        })

    trace = bool(os.environ.get("MHA_TRACE"))
    res = run_bass_kernel_spmd(nc, in_maps, core_ids=list(range(N_CORES)),
                               trace=trace)
    LAST_RESULT = res

    out = np.empty((B, T, C), np.float32)
    for b in range(B):
        out[b] = res.results[2 * b]["y"] + res.results[2 * b + 1]["y"]
    out += bp[None, None, :]
    return out


# revision 16
# speedup vs baseline: 1.0201x; 1.0061x over previous
"""TRN2 Bass kernel for 16-head causal MHA (B=4, T=2048, C=2048), fp32 in/out.

Sharding: 8 cores = 4 batches x 2 head-groups (8 heads each).  Each core
computes q/k/v projections for its head group on its batch (tensor-parallel
column split of Wq/Wk/Wv), causal attention in the S^T layout, and a partial
output projection with the row slice of Wp.  The two head-group partials per
batch are summed on the host, plus the output bias.

Design (v7, bf16 datapath):
- All operand data is bf16; every matmul accumulates in fp32 PSUM.
- Startup: DMA emission is ordered by first consumption.  The sync queue
  carries only the Q/K weight columns (first two waves' stationaries split
  so the first LDWEIGHTS fires after 64KB); x^T chunks ride the scalar +
  gpsimd queues in consumption order; ones/tri/wvg/wp follow later.
- Q^T/K^T via e-interleaved wave pairs (8 PSUM banks) as before; V is
  computed for t-tiles 0..7 in the projection phase, while t-tiles 8..15
  are DEFERRED and emitted as PE filler inside the qg=0/qg=1 attention
  stretch (which is otherwise exp-latency bound).  x^T is split into two
  tiles (t<1024 / t>=1024) so only the high half (+wv weights) stays
  resident during that overlap window.
- Attention (S^T layout, multiplicative causal mask on diagonal blocks,
  packed variable-width score/attn matmuls, ones-matmul softmax
  denominator broadcast, single-pass DVE reciprocal, gpsimd normalize)
  is unchanged except: atn lives in two qg-major tiles (qg0/1, qg2/3),
  and out-projection D-blocks are interleaved WITHIN qg2/qg3 cgroups
  (between score pairs) so exp latency never stalls the PE.
- Output projection column groups drain og-major so the og-sliced Wp DMA
  (issued as SBUF frees up after the overlap window) arrives just in time;
  y tiles DMA out round-robin across the sync/scalar/gpsimd queues.
"""
import math
import os
from collections import deque

import ml_dtypes
import numpy as np

import concourse.bass as bass
import concourse.tile as tile
from concourse import bacc, mybir
from concourse.bass_utils import run_bass_kernel_spmd

f32 = mybir.dt.float32
bf16 = mybir.dt.bfloat16
AF = mybir.ActivationFunctionType
BF = ml_dtypes.bfloat16

N_CORES = 8
HD = 128                      # head dim

# results of the last run_bass_kernel_spmd call (for test harness profiling)
LAST_RESULT = None


def build_nc(T=2048, E=2048, D=1024, NOD=2048, TG=512, bias=False,
             num_devices=N_CORES):
    """Build + compile the per-core Bass program."""
    NH = D // HD              # heads per core
    EC = E // 128 + (1 if bias else 0)
    TC = T // 128             # 128-row tiles along T
    TGC = T // TG             # q-groups
    ODG = NOD // 512          # out-proj column groups
    VN = 512                  # v-projection moving width
    NVS = D // VN
    TLO = T // 2              # t-split point: xt_lo covers t<TLO
    TTLO = TLO // 128         # t-tiles in the low half (V in phase B)
    scale = 1.0 / math.sqrt(HD)

    nc = bacc.Bacc("TRN2", target_bir_lowering=False, debug=False,
                   num_devices=num_devices)

    xT_d = nc.dram_tensor("xT", [EC * 128, T], bf16, kind="ExternalInput")
    wq_d = nc.dram_tensor("wq", [D // 128, 128, EC * 128], bf16,
                          kind="ExternalInput")
    wk_d = nc.dram_tensor("wk", [D // 128, 128, EC * 128], bf16,
                          kind="ExternalInput")
    wv_d = nc.dram_tensor("wv", [NVS, 128, EC * VN], bf16,
                          kind="ExternalInput")
    wp_d = nc.dram_tensor("wp", [D, NOD], bf16, kind="ExternalInput")
    tri_d = nc.dram_tensor("tri", [128, 128], bf16, kind="ExternalInput")
    ones_d = nc.dram_tensor("ones", [128, 128], bf16, kind="ExternalInput")
    y_d = nc.dram_tensor("y", [T, NOD], f32, kind="ExternalOutput")

    with tile.TileContext(nc) as tc:
        # ---- pools (manual lifetime management, dual-sided allocator) ----
        persist = tc.alloc_tile_pool(name="persist", bufs=1)            # L
        wv_pool = tc.alloc_tile_pool(name="wvp", bufs=1)                # L
        xthi_pool = tc.alloc_tile_pool(name="xthi", bufs=1)             # L
        xtlo_pool = tc.alloc_tile_pool(name="xtlo", bufs=1)             # L
        wcol_pool = tc.alloc_tile_pool(name="wcola", bufs=3)            # L
        qk_psum = tc.alloc_tile_pool(name="qk_psum", bufs=8, space="PSUM")

        # persistent tiles: q^T/k^T in [d, t] layout (head h = 128-row
        # chunk h), v in natural [t, d] layout ([t%128, tt*D + d]).
        qt_all = persist.tile([128, NH * T], bf16)
        kt_all = persist.tile([128, NH * T], bf16)
        v_all = persist.tile([128, TC * D], bf16)
        ones_sb = persist.tile([128, 128], bf16)
        tri_sb = persist.tile([128, 128], bf16)
        scr = persist.tile([1, 1], f32)

        # ---------------- phase A+B: q/k/v projections ----------------
        xt_lo = xtlo_pool.tile([128, EC * TLO], bf16)
        xt_hi = xthi_pool.tile([128, EC * (T - TLO)], bf16)

        def xt_tg(e, tg):
            # moving slice for q-group tg (TG=512) of e-chunk e
            off = tg * TG
            if off + TG <= TLO:
                return xt_lo[:, e * TLO + off:e * TLO + off + TG]
            off -= TLO
            return xt_hi[:, e * (T - TLO) + off:e * (T - TLO) + off + TG]

        def xt_tt(e, tt):
            # stationary slice for t-tile tt of e-chunk e
            off = tt * 128
            if off + 128 <= TLO:
                return xt_lo[:, e * TLO + off:e * TLO + off + 128]
            off -= TLO
            return xt_hi[:, e * (T - TLO) + off:e * (T - TLO) + off + 128]

        dsts = (qt_all, kt_all)
        wds = (wq_d, wk_d)
        wpairs = [(w_i, dc) for w_i in range(2) for dc in range(D // 128)]

        def load_wcol(w_i, dc, split=False):
            wcol = wcol_pool.tile([128, EC * 128], bf16, tag="wcol",
                                  name=f"wcol_{w_i}_{dc}")
            if split:
                # first two e-slices land first so the first LDWEIGHTS
                # fires after 64KB instead of 512KB
                nc.sync.dma_start(wcol[:, 0:256], wds[w_i][dc][:, 0:256])
                nc.sync.dma_start(wcol[:, 256:], wds[w_i][dc][:, 256:])
            else:
                nc.sync.dma_start(wcol[:], wds[w_i][dc])
            return wcol

        # Critical-path DMA order: sync queue = wq0a, wk0a, wq0b, wk0b,
        # then its share of x^T, then the next wcols; the other x^T
        # chunks ride scalar+gpsimd round-robin.  The first two waves
        # consume e-chunks in ARRIVAL order (PSUM accumulation is
        # commutative) so no single late chunk stalls the PE.
        # x^T queue split: the sync queue is busy with 2MB of wcols during
        # the slow HBM ramp, so it only gets 2 late chunks; gpsimd/scalar
        # alternate the rest.
        y_set = sorted({EC // 3, 2 * EC // 3 + 1})
        rest = [e for e in range(EC) if e not in y_set]
        g_list = rest[0::2]
        s_list = rest[1::2]
        wcol_q = [load_wcol(*wpairs[0], split=True),
                  load_wcol(*wpairs[1], split=True)]
        for e in range(EC):
            for (dst, lo) in ((xt_lo[:, e * TLO:(e + 1) * TLO], 0),
                              (xt_hi[:, e * (T - TLO):(e + 1) * (T - TLO)],
                               TLO)):
                src = xT_d[e * 128:(e + 1) * 128, lo:lo + (TLO if lo == 0
                                                           else T - TLO)]
                if e in g_list:
                    nc.gpsimd.dma_start(out=dst, in_=src)
                elif e in s_list:
                    nc.scalar.dma_start(dst, src)
                else:
                    nc.sync.dma_start(dst, src)
        wcol_q.append(load_wcol(*wpairs[2], split=True))
        wcol_q.append(load_wcol(*wpairs[3], split=True))
        # ones/tri + wv weights on the scalar queue (first needed at V /
        # attention, long after its x^T share drains); dummy exp pulls
        # the ~2.7us ACT table load off the attention critical path.
        nc.scalar.dma_start(ones_sb[:], ones_d[:])
        nc.scalar.dma_start(tri_sb[:], tri_d[:])
        nc.scalar.activation(scr[:], ones_sb[0:1, 0:1], AF.Exp, scale=1.0)
        wvgs = []
        for dg in range(NVS):
            wvg = wv_pool.tile([128, EC * VN], bf16, tag=f"wvg{dg}",
                               name=f"wvg_{dg}")
            nc.scalar.dma_start(wvg[:], wv_d[dg])
            wvgs.append(wvg)

        # arrival rank of e-chunks: queue q's k-th chunk lands ~(k+1)
        # chunk-times in; the sync queue's are ~5 chunk-times apart
        # (behind 2MB of wcol halves).
        e_order = [e for _, e in sorted(
            [(k + 1.0, e) for k, e in enumerate(g_list)]
            + [(k + 1.01, e) for k, e in enumerate(s_list)]
            + [(6.5 + 5.0 * k, e) for k, e in enumerate(y_set)])]

        # Q^T / K^T: one (weight, d-chunk) per wave of 4 PSUM banks,
        # e-major inside the wave; 8 banks = two waves in flight.
        # The first TWO waves are e-interleaved so each arriving xT
        # chunk feeds 8 matmuls, matching the chunk DMA arrival rate.
        ngrp = T // TG

        def qk_wave_tiles(w_i, dc):
            return [qk_psum.tile([128, TG], f32, tag="abps",
                                 name=f"abps_{w_i}_{dc}_{tg}")
                    for tg in range(ngrp)]

        def qk_wave_mms(pss, wcol, e):
            for tg in range(ngrp):
                nc.tensor.matmul(
                    pss[tg][:],
                    wcol[:, e * 128:(e + 1) * 128],
                    xt_tg(e, tg),
                    start=(e == 0), stop=(e == EC - 1),
                )

        def qk_wave_copies(pss, w_i, dc):
            for tg in range(ngrp):
                nc.scalar.copy(
                    dsts[w_i][:, dc * T + tg * TG:dc * T + (tg + 1) * TG],
                    pss[tg][:])

        pss0 = qk_wave_tiles(*wpairs[0])
        pss1 = qk_wave_tiles(*wpairs[1])
        wcol0, wcol1 = wcol_q.pop(0), wcol_q.pop(0)
        for i, e in enumerate(e_order):
            st, sp = i == 0, i == EC - 1
            for tg in range(ngrp):
                for (pss, wcol) in ((pss0, wcol0), (pss1, wcol1)):
                    nc.tensor.matmul(
                        pss[tg][:], wcol[:, e * 128:(e + 1) * 128],
                        xt_tg(e, tg), start=st, stop=sp)
        qk_wave_copies(pss0, *wpairs[0])
        qk_wave_copies(pss1, *wpairs[1])
        for wi in range(2, len(wpairs)):
            w_i, dc = wpairs[wi]
            wcol = wcol_q.pop(0)
            if wi + 2 < len(wpairs):
                wcol_q.append(load_wcol(*wpairs[wi + 2]))
            pss = qk_wave_tiles(w_i, dc)
            for e in range(EC):
                qk_wave_mms(pss, wcol, e)
            qk_wave_copies(pss, w_i, dc)

        # V (natural [t, d] layout, full 512 moving width) for the LOW
        # t-tiles only; the high t-tiles are deferred into the attention
        # overlap below.  ACT is idle here so copies alternate ACT/DVE.
        for tt in range(TTLO):
            for dg in range(NVS):
                ps = qk_psum.tile([128, VN], f32, tag="abps",
                                  name=f"vps_{tt}_{dg}")
                for e in range(EC):
                    nc.tensor.matmul(
                        ps[:], xt_tt(e, tt),
                        wvgs[dg][:, e * VN:(e + 1) * VN],
                        start=(e == 0), stop=(e == EC - 1),
                    )
                dst = v_all[:, tt * D + dg * VN:tt * D + (dg + 1) * VN]
                if (tt * NVS + dg) % 2:
                    nc.vector.tensor_copy(dst, ps[:])
                else:
                    nc.scalar.copy(dst, ps[:])

        # ---------------- overlap window: free lo-half, open attention ----
        wcol_pool.release()
        xtlo_pool.release()
        qk_psum.release()

        v_psum = tc.alloc_tile_pool(name="v_psum", bufs=2, space="PSUM")
        s_psum = tc.alloc_tile_pool(name="s_psum", bufs=2, space="PSUM",
                                    side="right")
        a_psum = tc.alloc_tile_pool(name="a_psum", bufs=1, space="PSUM",
                                    side="right")
        d_psum = tc.alloc_tile_pool(name="d_psum", bufs=1, space="PSUM",
                                    side="right")
        cd1 = tc.alloc_tile_pool(name="cd1", bufs=1, side="right")
        pt_pool = tc.alloc_tile_pool(name="pt", bufs=3, side="right")
        sm_pool = tc.alloc_tile_pool(name="sm", bufs=2, side="right")

        # atn in qg-major layout: qg0/1 here, qg2/3 allocated post-overlap
        atn01 = cd1.tile([128, 2 * NH * TG], bf16)
        # Wp og-halves: og0/1 DMA can start as soon as cd1 exists (its
        # address range doesn't overlap the still-live xt_hi/wv tiles)
        wp_sb_lo = cd1.tile([128, NH * 2 * 512], bf16)
        nc.scalar.dma_start(
            wp_sb_lo.rearrange("p (hc og o) -> p hc og o", hc=NH, og=2),
            wp_d.rearrange("(hc p) (og o) -> p hc og o", p=128, o=512)
            [:, :, 0:2, :],
        )
        dsum_t = d_psum.tile([128, TG], f32)

        atn_hold = [atn01, None]   # atn23 filled in post-overlap

        def atn_slice(qg, h, lo, wdt):
            tile_ = atn_hold[qg // 2]
            base = (qg % 2) * NH * TG + h * TG + lo
            return tile_[:, base:base + wdt]

        # Deferred-V filler: emits V matmuls for t-tiles TTLO..TC-1 in
        # ~cycle-budgeted chunks between attention pieces.
        class VFiller:
            def __init__(self):
                self.items = [(tt, dg) for tt in range(TTLO, TC)
                              for dg in range(NVS)]
                self.idx = 0
                self.e = 0
                self.ps = None

            def done(self):
                return self.idx >= len(self.items)

            def take(self, ncy):
                n = (ncy + VN - 1) // VN
                while n > 0 and not self.done():
                    tt, dg = self.items[self.idx]
                    if self.ps is None:
                        self.ps = v_psum.tile([128, VN], f32, tag="vtps",
                                              name=f"vtail_{tt}_{dg}")
                        self.e = 0
                    e = self.e
                    nc.tensor.matmul(
                        self.ps[:], xt_tt(e, tt),
                        wvgs[dg][:, e * VN:(e + 1) * VN],
                        start=(e == 0), stop=(e == EC - 1),
                    )
                    self.e += 1
                    n -= 1
                    if self.e == EC:
                        dst = v_all[:, tt * D + dg * VN:
                                    tt * D + (dg + 1) * VN]
                        nc.vector.tensor_copy(dst, self.ps[:])
                        self.ps = None
                        self.idx += 1

            def drain(self):
                self.take(1 << 30)

        filler = VFiller()

        def emit_cgroup(qg, h, between=None):
            """Emit one (q-group, head) attention block.  `between(p)` is
            called after each pair's exp emission — PE filler goes there
            so the exp latency is always covered."""
            qbase = qg * TG
            npairs = 2 * (qg + 1)
            nk = 4 * (qg + 1)
            kc0 = qg * 4           # first diagonal k-chunk

            def pair_desc(p):
                # [(kc, soff, width, qoff)], exp width
                if p == npairs - 2:
                    return [(kc0, 0, 512, 0),
                            (kc0 + 1, 512, 384, 128)], 896
                if p == npairs - 1:
                    return [(kc0 + 2, 0, 256, 256),
                            (kc0 + 3, 256, 128, 384)], 384
                return [(2 * p, 0, 512, 0),
                        (2 * p + 1, 512, 512, 0)], 1024

            pts = [None] * npairs
            p_sum = pt_pool.tile([128, TG], bf16, tag="psacc",
                                 bufs=2, name=f"psacc_{qg}_{h}")

            def emit_av(p):
                parts, _ = pair_desc(p)
                p_t = pts[p]
                for (kc, soff, w, qoff) in parts:
                    nc.tensor.matmul(
                        atn_ps[:, qoff:qoff + w],
                        v_all[:, kc * D + h * HD:kc * D + (h + 1) * HD],
                        p_t[:, soff:soff + w],
                        start=(kc == 0), stop=(kc == nk - 1),
                    )
                for (kc, soff, w, qoff) in parts:
                    if kc == 0:
                        nc.vector.tensor_copy(p_sum[:], p_t[:, 0:TG])
                    else:
                        nc.vector.tensor_add(
                            p_sum[:, qoff:qoff + w],
                            p_sum[:, qoff:qoff + w],
                            p_t[:, soff:soff + w])

            atn_ps = a_psum.tile([128, TG], f32, tag="atn",
                                 name=f"atn_{qg}_{h}")
            for p in range(npairs):
                parts, expw = pair_desc(p)
                s_pair = s_psum.tile([128, 2 * TG], f32, tag="sp",
                                     name=f"sp_{qg}_{h}_{p}")
                packed = p == npairs - 1
                for pi, (kc, soff, w, qoff) in enumerate(parts):
                    nc.tensor.matmul(
                        s_pair[:, soff:soff + w],
                        kt_all[:, h * T + kc * 128:h * T + (kc + 1) * 128],
                        qt_all[:, h * T + qbase + qoff:h * T + qbase + 512],
                        start=(not packed or pi == 0),
                        stop=(not packed or pi == len(parts) - 1),
                    )
                p_t = pt_pool.tile([128, 2 * TG], bf16, tag="pt",
                                   name=f"pt_{qg}_{h}_{p}")
                pts[p] = p_t
                nc.scalar.activation(p_t[:, 0:expw], s_pair[:, 0:expw],
                                     AF.Exp, scale=scale)
                if p >= npairs - 2:
                    for (kc, soff, w, qoff) in parts:
                        nc.vector.tensor_mul(
                            p_t[:, soff:soff + 128],
                            p_t[:, soff:soff + 128],
                            tri_sb[:])
                if between is not None:
                    between(p)
                if p > 0:
                    emit_av(p - 1)
            emit_av(npairs - 1)
            slot = qg * NH + h
            atn_u = sm_pool.tile([128, TG], bf16, tag="atnu",
                                 name=f"atnu_{slot}")
            nc.scalar.copy(atn_u[:], atn_ps[:])

            def finalize():
                # ones[128,128]-matmul broadcasts the column-sums of
                # p_sum to every partition; single-pass DVE reciprocal;
                # gpsimd multiply into the atn tile.  Deferred past the
                # caller's interleaved PE work so the DVE p_sum chain
                # drains off the PE critical path.
                nc.tensor.matmul(dsum_t[:], ones_sb[:], p_sum[:],
                                 start=True, stop=True)
                recipB = sm_pool.tile([128, TG], f32, tag="rB",
                                      name=f"rB_{slot}")
                nc.vector.reciprocal_approx_fast(out=recipB[:],
                                                 in_=dsum_t[:])
                nc.gpsimd.tensor_mul(
                    atn_slice(qg, h, 0, TG), atn_u[:], recipB[:])

            return finalize

        # Overlap: qg=0 and qg=1 attention with deferred-V as PE filler.
        # Consumes 160 of the 256 deferred-V matmuls between pairs; the
        # remainder drains as one solid PE block before qg=2.
        for qg in range(2):
            for h in range(NH):
                fin = emit_cgroup(qg, h, between=lambda p: filler.take(1536))
                filler.take(3072)
                fin()
        filler.drain()

        # ---------------- phase C+D proper: qg2/3 + out-projection ----
        xthi_pool.release()
        wv_pool.release()
        v_psum.release()
        y_psum = tc.alloc_tile_pool(name="y_psum", bufs=2, space="PSUM")
        cd2 = tc.alloc_tile_pool(name="cd2", bufs=1, side="right")
        yst_pool = tc.alloc_tile_pool(name="yst", bufs=2, side="right")
        atn23 = cd2.tile([128, 2 * NH * TG], bf16)
        atn_hold[1] = atn23
        wp_sb_hi = cd2.tile([128, NH * 2 * 512], bf16)
        nc.sync.dma_start(
            wp_sb_hi.rearrange("p (hc og o) -> p hc og o", hc=NH, og=2),
            wp_d.rearrange("(hc p) (og o) -> p hc og o", p=128, o=512)
            [:, :, 2:4, :],
        )

        ndb = [0]

        def emit_dblock(tt, og):
            qg, ttq = tt // 4, tt % 4
            wp_t = wp_sb_lo if og < 2 else wp_sb_hi
            ogl = og % 2
            ps = y_psum.tile([128, 512], f32, tag="yps",
                             name=f"yps_{tt}_{og}")
            for hc in range(NH):
                nc.tensor.matmul(
                    ps[:],
                    atn_slice(qg, hc, ttq * 128, 128),
                    wp_t[:, (hc * 2 + ogl) * 512:(hc * 2 + ogl + 1) * 512],
                    start=(hc == 0), stop=(hc == NH - 1),
                )
            yst = yst_pool.tile([128, 512], f32, tag="yst",
                                name=f"yst_{tt}_{og}")
            if ndb[0] % 2:
                nc.vector.tensor_copy(yst[:], ps[:])
            else:
                nc.scalar.copy(yst[:], ps[:])
            dst = y_d[tt * 128:(tt + 1) * 128, og * 512:(og + 1) * 512]
            m = ndb[0] % 3
            ndb[0] += 1
            if m == 0:
                nc.sync.dma_start(dst, yst[:])
            elif m == 1:
                nc.scalar.dma_start(dst, yst[:])
            else:
                nc.gpsimd.dma_start(out=dst, in_=yst[:])

        # dblock queue: og-major within each qg batch so the Wp og-halves
        # arrive in drain order
        dq = deque()
        for qg in range(2):
            for og in range(ODG):
                for tt in range(qg * 4, qg * 4 + 4):
                    dq.append((tt, og))

        def drain(n):
            for _ in range(n):
                if dq:
                    emit_dblock(*dq.popleft())

        # qg2: 3 dblocks inside each cgroup
        for h in range(NH):
            fin = emit_cgroup(
                2, h, between=lambda p: drain(1) if p in (1, 3, 5) else None)
            fin()
        for og in range(ODG):
            for tt in range(8, 12):
                dq.append((tt, og))
        # qg3: 3 dblocks inside each cgroup
        for h in range(NH):
            fin = emit_cgroup(
                3, h, between=lambda p: drain(1) if p in (1, 3, 5) else None)
            fin()
        while dq:
            emit_dblock(*dq.popleft())

        def emit_dtile(tt):
            # tail-only: all 4 og chains of one t-tile into a single
            # [128, 2048] staging tile and ONE 1MB y DMA — fewer DMA
            # completion semaphores to drain at kernel teardown
            qg = tt // 4
            yst = yst_pool.tile([128, NOD], f32, tag="ytile",
                                name=f"ytile_{tt}")
            for og in range(ODG):
                ps = y_psum.tile([128, 512], f32, tag="yps",
                                 name=f"yps_{tt}_{og}")
                wp_t = wp_sb_lo if og < 2 else wp_sb_hi
                ogl = og % 2
                for hc in range(NH):
                    nc.tensor.matmul(
                        ps[:],
                        atn_slice(qg, hc, (tt % 4) * 128, 128),
                        wp_t[:, (hc * 2 + ogl) * 512:
                             (hc * 2 + ogl + 1) * 512],
                        start=(hc == 0), stop=(hc == NH - 1),
                    )
                if og % 2:
                    nc.vector.tensor_copy(
                        yst[:, og * 512:(og + 1) * 512], ps[:])
                else:
                    nc.scalar.copy(yst[:, og * 512:(og + 1) * 512], ps[:])
            m = tt % 3
            dst = y_d[tt * 128:(tt + 1) * 128, :]
            if m == 0:
                nc.sync.dma_start(dst, yst[:])
            elif m == 1:
                nc.scalar.dma_start(dst, yst[:])
            else:
                nc.gpsimd.dma_start(out=dst, in_=yst[:])

        for tt in range(12, 16):
            emit_dtile(tt)

        # release remaining pools (LIFO per side)
        y_psum.release()
        d_psum.release()
        a_psum.release()
        s_psum.release()
        yst_pool.release()
        cd2.release()
        sm_pool.release()
        pt_pool.release()
        cd1.release()
        persist.release()

    nc.compile()
    return nc


def _augment(mat, bias_row, pad_to):
    """Append [bias_row; zeros] below mat so it has pad_to rows."""
    extra = np.zeros((pad_to - mat.shape[0], mat.shape[1]), np.float32)
    extra[0] = bias_row
    return np.concatenate([mat, extra], axis=0)


def _swizzle_qk(w, EC):
    """[EC*128, D] -> [D//128, 128, EC*128]: per-wave slice partition-major
    so its DMA moves in 4KB packets."""
    D = w.shape[1]
    return np.ascontiguousarray(
        w.reshape(EC, 128, D // 128, 128).transpose(2, 1, 0, 3)
        .reshape(D // 128, 128, EC * 128).astype(BF))


def _swizzle_v(w, EC, VN=512):
    """[EC*128, D] -> [D//VN, 128, EC*VN] partition-major."""
    D = w.shape[1]
    return np.ascontiguousarray(
        w.reshape(EC, 128, D // VN, VN).transpose(2, 1, 0, 3)
        .reshape(D // VN, 128, EC * VN).astype(BF))


_NC_CACHE = {}


def _get_nc(bias):
    if bias not in _NC_CACHE:
        _NC_CACHE[bias] = build_nc(bias=bias)
    return _NC_CACHE[bias]


def kernel(x, Wq, bq, Wk, bk, Wv, bv, Wp, bp):
    global LAST_RESULT
    x = np.ascontiguousarray(np.asarray(x, np.float32))
    Wq, bq = np.asarray(Wq, np.float32), np.asarray(bq, np.float32)
    Wk, bk = np.asarray(Wk, np.float32), np.asarray(bk, np.float32)
    Wv, bv = np.asarray(Wv, np.float32), np.asarray(bv, np.float32)
    Wp, bp = np.asarray(Wp, np.float32), np.asarray(bp, np.float32)

    B, T, C = x.shape
    assert (B, T, C) == (4, 2048, 2048), (B, T, C)
    D = 1024  # head-group width: 8 heads per core
    bias = bool(np.any(bq) or np.any(bk) or np.any(bv))
    nc = _get_nc(bias)

    kk = np.arange(128)[:, None]
    qq = np.arange(128)[None, :]
    tri = (kk <= qq).astype(BF)
    ones = np.ones((128, 128), BF)
    Ep = C + 128 if bias else C

    in_maps = []
    for c in range(N_CORES):
        b, g = c // 2, c % 2
        xt = x[b].T
        wq_g = Wq[:, g * D:(g + 1) * D]
        wk_g = Wk[:, g * D:(g + 1) * D]
        wv_g = Wv[:, g * D:(g + 1) * D]
        if bias:
            xt = _augment(xt, np.ones(T, np.float32), Ep)
            wq_g = _augment(wq_g, bq[g * D:(g + 1) * D], Ep)
            wk_g = _augment(wk_g, bk[g * D:(g + 1) * D], Ep)
            wv_g = _augment(wv_g, bv[g * D:(g + 1) * D], Ep)
        EC = Ep // 128
        in_maps.append({
            "xT": np.ascontiguousarray(xt.astype(BF)),
            "wq": _swizzle_qk(wq_g, EC),
            "wk": _swizzle_qk(wk_g, EC),
            "wv": _swizzle_v(wv_g, EC),
            "wp": np.ascontiguousarray(Wp[g * D:(g + 1) * D, :].astype(BF)),
            "tri": tri,
            "ones": ones,
        })

    trace = bool(os.environ.get("MHA_TRACE"))
    res = run_bass_kernel_spmd(nc, in_maps, core_ids=list(range(N_CORES)),
                               trace=trace)
    LAST_RESULT = res

    out = np.empty((B, T, C), np.float32)
    for b in range(B):
        out[b] = res.results[2 * b]["y"] + res.results[2 * b + 1]["y"]
    out += bp[None, None, :]
    return out


# revision 18
# speedup vs baseline: 1.0270x; 1.0068x over previous
"""TRN2 Bass kernel for 16-head causal MHA (B=4, T=2048, C=2048), fp32 in/out.

Sharding: 8 cores = 4 batches x 2 head-groups (8 heads each).  Each core
computes q/k/v projections for its head group on its batch (tensor-parallel
column split of Wq/Wk/Wv), causal attention in the S^T layout, and a partial
output projection with the row slice of Wp.  The two head-group partials per
batch are summed on the host, plus the output bias.

Design (v7, bf16 datapath):
- All operand data is bf16; every matmul accumulates in fp32 PSUM.
- Startup: DMA emission is ordered by first consumption.  The sync queue
  carries only the Q/K weight columns (first two waves' stationaries split
  so the first LDWEIGHTS fires after 64KB); x^T chunks ride the scalar +
  gpsimd queues in consumption order; ones/tri/wvg/wp follow later.
- Q^T/K^T via e-interleaved wave pairs (8 PSUM banks) as before; V is
  computed for t-tiles 0..7 in the projection phase, while t-tiles 8..15
  are DEFERRED and emitted as PE filler inside the qg=0/qg=1 attention
  stretch (which is otherwise exp-latency bound).  x^T is split into two
  tiles (t<1024 / t>=1024) so only the high half (+wv weights) stays
  resident during that overlap window.
- Attention (S^T layout, multiplicative causal mask on diagonal blocks,
  packed variable-width score/attn matmuls, ones-matmul softmax
  denominator broadcast, single-pass DVE reciprocal, gpsimd normalize)
  is unchanged except: atn lives in two qg-major tiles (qg0/1, qg2/3),
  and out-projection D-blocks are interleaved WITHIN qg2/qg3 cgroups
  (between score pairs) so exp latency never stalls the PE.
- Output projection column groups drain og-major so the og-sliced Wp DMA
  (issued as SBUF frees up after the overlap window) arrives just in time;
  y tiles DMA out round-robin across the sync/scalar/gpsimd queues.
"""
import math
import os
from collections import deque

import ml_dtypes
import numpy as np

import concourse.bass as bass
import concourse.tile as tile
from concourse import bacc, mybir
from concourse.bass_utils import run_bass_kernel_spmd

f32 = mybir.dt.float32
bf16 = mybir.dt.bfloat16
AF = mybir.ActivationFunctionType
BF = ml_dtypes.bfloat16

N_CORES = 8
HD = 128                      # head dim

# results of the last run_bass_kernel_spmd call (for test harness profiling)
LAST_RESULT = None


def build_nc(T=2048, E=2048, D=1024, NOD=2048, TG=512, bias=False,
             num_devices=N_CORES):
    """Build + compile the per-core Bass program."""
    NH = D // HD              # heads per core
    EC = E // 128 + (1 if bias else 0)
    TC = T // 128             # 128-row tiles along T
    TGC = T // TG             # q-groups
    ODG = NOD // 512          # out-proj column groups
    VN = 512                  # v-projection moving width
    NVS = D // VN
    TLO = T // 2              # t-split point: xt_lo covers t<TLO
    TTLO = TLO // 128         # t-tiles in the low half (V in phase B)
    scale = 1.0 / math.sqrt(HD)

    nc = bacc.Bacc("TRN2", target_bir_lowering=False, debug=False,
                   num_devices=num_devices)

    xT_d = nc.dram_tensor("xT", [EC * 128, T], bf16, kind="ExternalInput")
    wq_d = nc.dram_tensor("wq", [D // 128, 128, EC * 128], bf16,
                          kind="ExternalInput")
    wk_d = nc.dram_tensor("wk", [D // 128, 128, EC * 128], bf16,
                          kind="ExternalInput")
    wv_d = nc.dram_tensor("wv", [NVS, 128, EC * VN], bf16,
                          kind="ExternalInput")
    wp_d = nc.dram_tensor("wp", [D, NOD], bf16, kind="ExternalInput")
    tri_d = nc.dram_tensor("tri", [128, 128], bf16, kind="ExternalInput")
    ones_d = nc.dram_tensor("ones", [128, 128], bf16, kind="ExternalInput")
    y_d = nc.dram_tensor("y", [T, NOD], f32, kind="ExternalOutput")

    with tile.TileContext(nc) as tc:
        # ---- pools (manual lifetime management, dual-sided allocator) ----
        persist = tc.alloc_tile_pool(name="persist", bufs=1)            # L
        wv_pool = tc.alloc_tile_pool(name="wvp", bufs=1)                # L
        xthi_pool = tc.alloc_tile_pool(name="xthi", bufs=1)             # L
        xtlo_pool = tc.alloc_tile_pool(name="xtlo", bufs=1)             # L
        wcol_pool = tc.alloc_tile_pool(name="wcola", bufs=3)            # L
        qk_psum = tc.alloc_tile_pool(name="qk_psum", bufs=8, space="PSUM")

        # persistent tiles: q^T/k^T in [d, t] layout (head h = 128-row
        # chunk h), v in natural [t, d] layout ([t%128, tt*D + d]).
        qt_all = persist.tile([128, NH * T], bf16)
        kt_all = persist.tile([128, NH * T], bf16)
        v_all = persist.tile([128, TC * D], bf16)
        ones_sb = persist.tile([128, 128], bf16)
        tri_sb = persist.tile([128, 128], bf16)
        scr = persist.tile([1, 1], f32)

        # ---------------- phase A+B: q/k/v projections ----------------
        xt_lo = xtlo_pool.tile([128, EC * TLO], bf16)
        xt_hi = xthi_pool.tile([128, EC * (T - TLO)], bf16)

        def xt_tg(e, tg):
            # moving slice for q-group tg (TG=512) of e-chunk e
            off = tg * TG
            if off + TG <= TLO:
                return xt_lo[:, e * TLO + off:e * TLO + off + TG]
            off -= TLO
            return xt_hi[:, e * (T - TLO) + off:e * (T - TLO) + off + TG]

        def xt_tt(e, tt):
            # stationary slice for t-tile tt of e-chunk e
            off = tt * 128
            if off + 128 <= TLO:
                return xt_lo[:, e * TLO + off:e * TLO + off + 128]
            off -= TLO
            return xt_hi[:, e * (T - TLO) + off:e * (T - TLO) + off + 128]

        dsts = (qt_all, kt_all)
        wds = (wq_d, wk_d)
        wpairs = [(w_i, dc) for w_i in range(2) for dc in range(D // 128)]

        def load_wcol(w_i, dc, split=False):
            wcol = wcol_pool.tile([128, EC * 128], bf16, tag="wcol",
                                  name=f"wcol_{w_i}_{dc}")
            if split:
                # first two e-slices land first so the first LDWEIGHTS
                # fires after 64KB instead of 512KB
                nc.sync.dma_start(wcol[:, 0:256], wds[w_i][dc][:, 0:256])
                nc.sync.dma_start(wcol[:, 256:], wds[w_i][dc][:, 256:])
            else:
                nc.sync.dma_start(wcol[:], wds[w_i][dc])
            return wcol

        # Critical-path DMA order: sync queue = wq0a, wk0a, wq0b, wk0b,
        # then its share of x^T, then the next wcols; the other x^T
        # chunks ride scalar+gpsimd round-robin.  The first two waves
        # consume e-chunks in ARRIVAL order (PSUM accumulation is
        # commutative) so no single late chunk stalls the PE.
        # x^T queue split: the sync queue is busy with 2MB of wcols during
        # the slow HBM ramp, so it only gets 2 late chunks; gpsimd/scalar
        # alternate the rest.
        y_set = sorted({EC // 3, 2 * EC // 3 + 1})
        rest = [e for e in range(EC) if e not in y_set]
        g_list = rest[0::2]
        s_list = rest[1::2]
        wcol_q = [load_wcol(*wpairs[0], split=True),
                  load_wcol(*wpairs[1], split=True)]
        for e in range(EC):
            for (dst, lo) in ((xt_lo[:, e * TLO:(e + 1) * TLO], 0),
                              (xt_hi[:, e * (T - TLO):(e + 1) * (T - TLO)],
                               TLO)):
                src = xT_d[e * 128:(e + 1) * 128, lo:lo + (TLO if lo == 0
                                                           else T - TLO)]
                if e in g_list:
                    nc.gpsimd.dma_start(out=dst, in_=src)
                elif e in s_list:
                    nc.scalar.dma_start(dst, src)
                else:
                    nc.sync.dma_start(dst, src)
        wcol_q.append(load_wcol(*wpairs[2], split=True))
        wcol_q.append(load_wcol(*wpairs[3], split=True))
        # ones/tri + wv weights on the scalar queue (first needed at V /
        # attention, long after its x^T share drains); dummy exp pulls
        # the ~2.7us ACT table load off the attention critical path.
        nc.scalar.dma_start(ones_sb[:], ones_d[:])
        nc.scalar.dma_start(tri_sb[:], tri_d[:])
        nc.scalar.activation(scr[:], ones_sb[0:1, 0:1], AF.Exp, scale=1.0)
        wvgs = []
        for dg in range(NVS):
            wvg = wv_pool.tile([128, EC * VN], bf16, tag=f"wvg{dg}",
                               name=f"wvg_{dg}")
            nc.scalar.dma_start(wvg[:], wv_d[dg])
            wvgs.append(wvg)

        # arrival rank of e-chunks: queue q's k-th chunk lands ~(k+1)
        # chunk-times in; the sync queue's are ~5 chunk-times apart
        # (behind 2MB of wcol halves).
        e_order = [e for _, e in sorted(
            [(k + 1.0, e) for k, e in enumerate(g_list)]
            + [(k + 1.01, e) for k, e in enumerate(s_list)]
            + [(6.5 + 5.0 * k, e) for k, e in enumerate(y_set)])]

        # Q^T / K^T: one (weight, d-chunk) per wave of 4 PSUM banks,
        # e-major inside the wave; 8 banks = two waves in flight.
        # The first TWO waves are e-interleaved so each arriving xT
        # chunk feeds 8 matmuls, matching the chunk DMA arrival rate.
        ngrp = T // TG

        def qk_wave_tiles(w_i, dc):
            return [qk_psum.tile([128, TG], f32, tag="abps",
                                 name=f"abps_{w_i}_{dc}_{tg}")
                    for tg in range(ngrp)]

        def qk_wave_mms(pss, wcol, e):
            for tg in range(ngrp):
                nc.tensor.matmul(
                    pss[tg][:],
                    wcol[:, e * 128:(e + 1) * 128],
                    xt_tg(e, tg),
                    start=(e == 0), stop=(e == EC - 1),
                )

        def qk_wave_copies(pss, w_i, dc):
            for tg in range(ngrp):
                nc.scalar.copy(
                    dsts[w_i][:, dc * T + tg * TG:dc * T + (tg + 1) * TG],
                    pss[tg][:])

        pss0 = qk_wave_tiles(*wpairs[0])
        pss1 = qk_wave_tiles(*wpairs[1])
        wcol0, wcol1 = wcol_q.pop(0), wcol_q.pop(0)
        for i, e in enumerate(e_order):
            st, sp = i == 0, i == EC - 1
            for tg in range(ngrp):
                for (pss, wcol) in ((pss0, wcol0), (pss1, wcol1)):
                    nc.tensor.matmul(
                        pss[tg][:], wcol[:, e * 128:(e + 1) * 128],
                        xt_tg(e, tg), start=st, stop=sp)
        qk_wave_copies(pss0, *wpairs[0])
        qk_wave_copies(pss1, *wpairs[1])
        for wi in range(2, len(wpairs)):
            w_i, dc = wpairs[wi]
            wcol = wcol_q.pop(0)
            if wi + 2 < len(wpairs):
                wcol_q.append(load_wcol(*wpairs[wi + 2]))
            pss = qk_wave_tiles(w_i, dc)
            for e in range(EC):
                qk_wave_mms(pss, wcol, e)
            qk_wave_copies(pss, w_i, dc)

        # V (natural [t, d] layout, full 512 moving width) for the LOW
        # t-tiles only; the high t-tiles are deferred into the attention
        # overlap below.  ACT is idle here so copies alternate ACT/DVE.
        for tt in range(TTLO):
            for dg in range(NVS):
                ps = qk_psum.tile([128, VN], f32, tag="abps",
                                  name=f"vps_{tt}_{dg}")
                for e in range(EC):
                    nc.tensor.matmul(
                        ps[:], xt_tt(e, tt),
                        wvgs[dg][:, e * VN:(e + 1) * VN],
                        start=(e == 0), stop=(e == EC - 1),
                    )
                dst = v_all[:, tt * D + dg * VN:tt * D + (dg + 1) * VN]
                if (tt * NVS + dg) % 2:
                    nc.vector.tensor_copy(dst, ps[:])
                else:
                    nc.scalar.copy(dst, ps[:])

        # ---------------- overlap window: free lo-half, open attention ----
        wcol_pool.release()
        xtlo_pool.release()
        qk_psum.release()

        v_psum = tc.alloc_tile_pool(name="v_psum", bufs=2, space="PSUM")
        s_psum = tc.alloc_tile_pool(name="s_psum", bufs=2, space="PSUM",
                                    side="right")
        a_psum = tc.alloc_tile_pool(name="a_psum", bufs=1, space="PSUM",
                                    side="right")
        d_psum = tc.alloc_tile_pool(name="d_psum", bufs=1, space="PSUM",
                                    side="right")
        cd1 = tc.alloc_tile_pool(name="cd1", bufs=1, side="right")
        pt_pool = tc.alloc_tile_pool(name="pt", bufs=3, side="right")
        sm_pool = tc.alloc_tile_pool(name="sm", bufs=2, side="right")

        # atn in qg-major layout: qg0/1 here, qg2/3 allocated post-overlap
        atn01 = cd1.tile([128, 2 * NH * TG], bf16)
        # Wp og-halves: og0/1 DMA can start as soon as cd1 exists (its
        # address range doesn't overlap the still-live xt_hi/wv tiles)
        wp_sb_lo = cd1.tile([128, NH * 2 * 512], bf16)
        nc.scalar.dma_start(
            wp_sb_lo.rearrange("p (hc og o) -> p hc og o", hc=NH, og=2),
            wp_d.rearrange("(hc p) (og o) -> p hc og o", p=128, o=512)
            [:, :, 0:2, :],
        )
        dsum_t = d_psum.tile([128, TG], f32)

        atn_hold = [atn01, None]   # atn23 filled in post-overlap

        def atn_slice(qg, h, lo, wdt):
            tile_ = atn_hold[qg // 2]
            base = (qg % 2) * NH * TG + h * TG + lo
            return tile_[:, base:base + wdt]

        # Deferred-V filler: emits V matmuls for t-tiles TTLO..TC-1 in
        # ~cycle-budgeted chunks between attention pieces.
        class VFiller:
            def __init__(self):
                self.items = [(tt, dg) for tt in range(TTLO, TC)
                              for dg in range(NVS)]
                self.idx = 0
                self.e = 0
                self.ps = None

            def done(self):
                return self.idx >= len(self.items)

            def take(self, ncy):
                n = (ncy + VN - 1) // VN
                while n > 0 and not self.done():
                    tt, dg = self.items[self.idx]
                    if self.ps is None:
                        self.ps = v_psum.tile([128, VN], f32, tag="vtps",
                                              name=f"vtail_{tt}_{dg}")
                        self.e = 0
                    e = self.e
                    nc.tensor.matmul(
                        self.ps[:], xt_tt(e, tt),
                        wvgs[dg][:, e * VN:(e + 1) * VN],
                        start=(e == 0), stop=(e == EC - 1),
                    )
                    self.e += 1
                    n -= 1
                    if self.e == EC:
                        dst = v_all[:, tt * D + dg * VN:
                                    tt * D + (dg + 1) * VN]
                        nc.vector.tensor_copy(dst, self.ps[:])
                        self.ps = None
                        self.idx += 1

            def drain(self):
                self.take(1 << 30)

        filler = VFiller()

        def emit_cgroup(qg, h, between=None):
            """Emit one (q-group, head) attention block.  `between(p)` is
            called after each pair's exp emission — PE filler goes there
            so the exp latency is always covered."""
            qbase = qg * TG
            npairs = 2 * (qg + 1)
            nk = 4 * (qg + 1)
            kc0 = qg * 4           # first diagonal k-chunk

            def pair_desc(p):
                # [(kc, soff, width, qoff)], exp width
                if p == npairs - 2:
                    return [(kc0, 0, 512, 0),
                            (kc0 + 1, 512, 384, 128)], 896
                if p == npairs - 1:
                    return [(kc0 + 2, 0, 256, 256),
                            (kc0 + 3, 256, 128, 384)], 384
                return [(2 * p, 0, 512, 0),
                        (2 * p + 1, 512, 512, 0)], 1024

            pts = [None] * npairs
            p_sum = pt_pool.tile([128, TG], bf16, tag="psacc",
                                 bufs=2, name=f"psacc_{qg}_{h}")

            def emit_av(p):
                parts, _ = pair_desc(p)
                p_t = pts[p]
                for (kc, soff, w, qoff) in parts:
                    nc.tensor.matmul(
                        atn_ps[:, qoff:qoff + w],
                        v_all[:, kc * D + h * HD:kc * D + (h + 1) * HD],
                        p_t[:, soff:soff + w],
                        start=(kc == 0), stop=(kc == nk - 1),
                    )
                for (kc, soff, w, qoff) in parts:
                    if kc == 0:
                        nc.vector.tensor_copy(p_sum[:], p_t[:, 0:TG])
                    else:
                        nc.vector.tensor_add(
                            p_sum[:, qoff:qoff + w],
                            p_sum[:, qoff:qoff + w],
                            p_t[:, soff:soff + w])

            atn_ps = a_psum.tile([128, TG], f32, tag="atn",
                                 name=f"atn_{qg}_{h}")
            for p in range(npairs):
                parts, expw = pair_desc(p)
                s_pair = s_psum.tile([128, 2 * TG], f32, tag="sp",
                                     name=f"sp_{qg}_{h}_{p}")
                packed = p == npairs - 1
                for pi, (kc, soff, w, qoff) in enumerate(parts):
                    nc.tensor.matmul(
                        s_pair[:, soff:soff + w],
                        kt_all[:, h * T + kc * 128:h * T + (kc + 1) * 128],
                        qt_all[:, h * T + qbase + qoff:h * T + qbase + 512],
                        start=(not packed or pi == 0),
                        stop=(not packed or pi == len(parts) - 1),
                    )
                p_t = pt_pool.tile([128, 2 * TG], bf16, tag="pt",
                                   name=f"pt_{qg}_{h}_{p}")
                pts[p] = p_t
                nc.scalar.activation(p_t[:, 0:expw], s_pair[:, 0:expw],
                                     AF.Exp, scale=scale)
                if p >= npairs - 2:
                    for (kc, soff, w, qoff) in parts:
                        nc.vector.tensor_mul(
                            p_t[:, soff:soff + 128],
                            p_t[:, soff:soff + 128],
                            tri_sb[:])
                if between is not None:
                    between(p)
                if p > 0:
                    emit_av(p - 1)
            emit_av(npairs - 1)
            slot = qg * NH + h
            atn_u = sm_pool.tile([128, TG], bf16, tag="atnu",
                                 name=f"atnu_{slot}")
            nc.scalar.copy(atn_u[:], atn_ps[:])

            def finalize():
                # ones[128,128]-matmul broadcasts the column-sums of
                # p_sum to every partition; single-pass DVE reciprocal;
                # gpsimd multiply into the atn tile.  Deferred past the
                # caller's interleaved PE work so the DVE p_sum chain
                # drains off the PE critical path.
                nc.tensor.matmul(dsum_t[:], ones_sb[:], p_sum[:],
                                 start=True, stop=True)
                recipB = sm_pool.tile([128, TG], f32, tag="rB",
                                      name=f"rB_{slot}")
                nc.vector.reciprocal_approx_fast(out=recipB[:],
                                                 in_=dsum_t[:])
                nc.gpsimd.tensor_mul(
                    atn_slice(qg, h, 0, TG), atn_u[:], recipB[:])

            return finalize

        # Overlap: qg=0 and qg=1 attention with deferred-V as PE filler.
        # Consumes 160 of the 256 deferred-V matmuls between pairs; the
        # remainder drains as one solid PE block before qg=2.
        for qg in range(2):
            for h in range(NH):
                fin = emit_cgroup(qg, h, between=lambda p: filler.take(1536))
                filler.take(4096 if qg == 0 else 3072)
                fin()
        filler.drain()

        # ---------------- phase C+D proper: qg2/3 + out-projection ----
        xthi_pool.release()
        wv_pool.release()
        v_psum.release()
        y_psum = tc.alloc_tile_pool(name="y_psum", bufs=2, space="PSUM")
        cd2 = tc.alloc_tile_pool(name="cd2", bufs=1, side="right")
        yst_pool = tc.alloc_tile_pool(name="yst", bufs=2, side="right")
        atn23 = cd2.tile([128, 2 * NH * TG], bf16)
        atn_hold[1] = atn23
        wp_sb_hi = cd2.tile([128, NH * 2 * 512], bf16)
        nc.sync.dma_start(
            wp_sb_hi.rearrange("p (hc og o) -> p hc og o", hc=NH, og=2),
            wp_d.rearrange("(hc p) (og o) -> p hc og o", p=128, o=512)
            [:, :, 2:4, :],
        )

        ndb = [0]

        def emit_dblock(tt, og):
            qg, ttq = tt // 4, tt % 4
            wp_t = wp_sb_lo if og < 2 else wp_sb_hi
            ogl = og % 2
            ps = y_psum.tile([128, 512], f32, tag="yps",
                             name=f"yps_{tt}_{og}")
            for hc in range(NH):
                nc.tensor.matmul(
                    ps[:],
                    atn_slice(qg, hc, ttq * 128, 128),
                    wp_t[:, (hc * 2 + ogl) * 512:(hc * 2 + ogl + 1) * 512],
                    start=(hc == 0), stop=(hc == NH - 1),
                )
            yst = yst_pool.tile([128, 512], f32, tag="yst",
                                name=f"yst_{tt}_{og}")
            if ndb[0] % 2:
                nc.vector.tensor_copy(yst[:], ps[:])
            else:
                nc.scalar.copy(yst[:], ps[:])
            dst = y_d[tt * 128:(tt + 1) * 128, og * 512:(og + 1) * 512]
            m = ndb[0] % 3
            ndb[0] += 1
            if m == 0:
                nc.sync.dma_start(dst, yst[:])
            elif m == 1:
                nc.scalar.dma_start(dst, yst[:])
            else:
                nc.gpsimd.dma_start(out=dst, in_=yst[:])

        # dblock queue: og-major within each qg batch so the Wp og-halves
        # arrive in drain order
        dq = deque()
        for qg in range(2):
            for og in range(ODG):
                for tt in range(qg * 4, qg * 4 + 4):
                    dq.append((tt, og))

        def drain(n):
            for _ in range(n):
                if dq:
                    emit_dblock(*dq.popleft())

        # qg2: 3 dblocks inside each cgroup
        for h in range(NH):
            fin = emit_cgroup(
                2, h, between=lambda p: drain(1) if p in (1, 3, 5) else None)
            fin()
        for og in range(ODG):
            for tt in range(8, 12):
                dq.append((tt, og))
        # qg3: 3 dblocks inside each cgroup (p=6 slot covers the last
        # pairs' exp latency)
        for h in range(NH):
            fin = emit_cgroup(
                3, h, between=lambda p: drain(1) if p in (1, 3, 6) else None)
            fin()
        while dq:
            emit_dblock(*dq.popleft())

        def emit_dtile(tt):
            # tail-only: all 4 og chains of one t-tile into a single
            # [128, 2048] staging tile and ONE 1MB y DMA — fewer DMA
            # completion semaphores to drain at kernel teardown
            qg = tt // 4
            yst = yst_pool.tile([128, NOD], f32, tag="ytile",
                                name=f"ytile_{tt}")
            for og in range(ODG):
                ps = y_psum.tile([128, 512], f32, tag="yps",
                                 name=f"yps_{tt}_{og}")
                wp_t = wp_sb_lo if og < 2 else wp_sb_hi
                ogl = og % 2
                for hc in range(NH):
                    nc.tensor.matmul(
                        ps[:],
                        atn_slice(qg, hc, (tt % 4) * 128, 128),
                        wp_t[:, (hc * 2 + ogl) * 512:
                             (hc * 2 + ogl + 1) * 512],
                        start=(hc == 0), stop=(hc == NH - 1),
                    )
                if og % 2:
                    nc.vector.tensor_copy(
                        yst[:, og * 512:(og + 1) * 512], ps[:])
                else:
                    nc.scalar.copy(yst[:, og * 512:(og + 1) * 512], ps[:])
            m = tt % 3
            dst = y_d[tt * 128:(tt + 1) * 128, :]
            if m == 0:
                nc.sync.dma_start(dst, yst[:])
            elif m == 1:
                nc.scalar.dma_start(dst, yst[:])
            else:
                nc.gpsimd.dma_start(out=dst, in_=yst[:])

        for tt in range(12, 16):
            emit_dtile(tt)

        # release remaining pools (LIFO per side)
        y_psum.release()
        d_psum.release()
        a_psum.release()
        s_psum.release()
        yst_pool.release()
        cd2.release()
        sm_pool.release()
        pt_pool.release()
        cd1.release()
        persist.release()

    nc.compile()
    return nc


def _augment(mat, bias_row, pad_to):
    """Append [bias_row; zeros] below mat so it has pad_to rows."""
    extra = np.zeros((pad_to - mat.shape[0], mat.shape[1]), np.float32)
    extra[0] = bias_row
    return np.concatenate([mat, extra], axis=0)


def _swizzle_qk(w, EC):
    """[EC*128, D] -> [D//128, 128, EC*128]: per-wave slice partition-major
    so its DMA moves in 4KB packets."""
    D = w.shape[1]
    return np.ascontiguousarray(
        w.reshape(EC, 128, D // 128, 128).transpose(2, 1, 0, 3)
        .reshape(D // 128, 128, EC * 128).astype(BF))


def _swizzle_v(w, EC, VN=512):
    """[EC*128, D] -> [D//VN, 128, EC*VN] partition-major."""
    D = w.shape[1]
    return np.ascontiguousarray(
        w.reshape(EC, 128, D // VN, VN).transpose(2, 1, 0, 3)
        .reshape(D // VN, 128, EC * VN).astype(BF))


_NC_CACHE = {}


def _get_nc(bias):
    if bias not in _NC_CACHE:
        _NC_CACHE[bias] = build_nc(bias=bias)
    return _NC_CACHE[bias]


def kernel(x, Wq, bq, Wk, bk, Wv, bv, Wp, bp):
    global LAST_RESULT
    x = np.ascontiguousarray(np.asarray(x, np.float32))
    Wq, bq = np.asarray(Wq, np.float32), np.asarray(bq, np.float32)
    Wk, bk = np.asarray(Wk, np.float32), np.asarray(bk, np.float32)
    Wv, bv = np.asarray(Wv, np.float32), np.asarray(bv, np.float32)
    Wp, bp = np.asarray(Wp, np.float32), np.asarray(bp, np.float32)

    B, T, C = x.shape
    assert (B, T, C) == (4, 2048, 2048), (B, T, C)
    D = 1024  # head-group width: 8 heads per core
    bias = bool(np.any(bq) or np.any(bk) or np.any(bv))
    nc = _get_nc(bias)

    kk = np.arange(128)[:, None]
    qq = np.arange(128)[None, :]
    tri = (kk <= qq).astype(BF)
    ones = np.ones((128, 128), BF)
    Ep = C + 128 if bias else C

    in_maps = []
    for c in range(N_CORES):
        b, g = c // 2, c % 2
        xt = x[b].T
        wq_g = Wq[:, g * D:(g + 1) * D]
        wk_g = Wk[:, g * D:(g + 1) * D]
        wv_g = Wv[:, g * D:(g + 1) * D]
        if bias:
            xt = _augment(xt, np.ones(T, np.float32), Ep)
            wq_g = _augment(wq_g, bq[g * D:(g + 1) * D], Ep)
            wk_g = _augment(wk_g, bk[g * D:(g + 1) * D], Ep)
            wv_g = _augment(wv_g, bv[g * D:(g + 1) * D], Ep)
        EC = Ep // 128
        in_maps.append({
            "xT": np.ascontiguousarray(xt.astype(BF)),
            "wq": _swizzle_qk(wq_g, EC),
            "wk": _swizzle_qk(wk_g, EC),
            "wv": _swizzle_v(wv_g, EC),
            "wp": np.ascontiguousarray(Wp[g * D:(g + 1) * D, :].astype(BF)),
            "tri": tri,
            "ones": ones,
        })

    trace = bool(os.environ.get("MHA_TRACE"))
    res = run_bass_kernel_spmd(nc, in_maps, core_ids=list(range(N_CORES)),
                               trace=trace)
    LAST_RESULT = res

    out = np.empty((B, T, C), np.float32)
    for b in range(B):
        out[b] = res.results[2 * b]["y"] + res.results[2 * b + 1]["y"]
    out += bp[None, None, :]
    return out
